# revision 71
# baseline (speedup 1.0000x reference)
"""AttentionPool Trainium2 Bass kernel (valid-token compaction).

Reference computation (per batch b):
    h      = tanh(x @ W1 + b1)          # [N, H*F]   (big matmul, bf16 on PE)
    scores = h @ W2 + b2                # [N, H]
    scores = where(mask, scores, -1e9)
    w      = softmax(scores, axis=N)    # per head
    pooled = w.T @ x                    # [H, D]
    y      = concat_h(pooled) @ Wout + bout   # [D]

Key structural ideas:
  - Invalid tokens get softmax weight 0 and contribute nothing to the
    output, so the host compacts each batch's valid tokens (~1024 of 2048
    at p=0.5) into a contiguous buffer and the big x@W1 matmul runs only
    on those (~2x less PE work). Batches are sorted by valid count and
    assigned to per-slot lengths NVS (exact, partial trailing 128-token
    chunks are fine), so every core does the same near-minimal work.
  - Padding inside a slot carries x=0: it cannot pollute the pooling sum,
    and the softmax denominator is computed as a validity-weighted PE
    reduction Z = sum_t v_t e_t (v in {1,0}), so no -1e9 mask tensor or
    max-shift exists anywhere. b2 cancels under softmax and is dropped.
  - All small-output reductions are shaped so the PE streams tiny free
    dims (matmul cost ~ out free size): the score dot emits [tok, H=4]
    tiles, pooling emits pooled^T [128d, H=4] tiles per (chunk, dc) that
    the DVE accumulates across chunks, and the output projection emits
    y^T [128d, B=4] columns. Everything lands pre-transposed for its
    consumer; the only PE transposes left are none.
  - The exp(scores) tiles are already token-major = the pooling lhsT.

Sharding: data-parallel over batch B=32 across 8 cores (4 batches/core,
count-sorted slot assignment, results un-permuted on the host). Weights
replicated. Matmuls bf16 (fp32 PSUM); softmax fp32 on the Act engine
(the exp_and_others act table holds both tanh and exp - no table loads).
|scores| <= ||W2||_1 ~ 18, so exp is safe without max-shift.

Pipelining (keeps the PE dense, which also keeps its p-state ramped):
  - per f-chunk mc: the 8 k-matmuls of chunk mc+1..mc+2 are issued before
    the score dot of chunk mc (depth-2 queue), hiding the tanh latency.
  - a sub-block's pooling/Z matmuls are deferred 3 sub-blocks down the
    instruction stream, when exp and the xn DMA have long finished.
  - W1 streams in 256-column half-quarters on two DMA queues (sync +
    gpsimd) paced to the first chunk's interleaved sub-pair consumption;
    xT prefetches 2 chunks ahead; xn rides the gpsimd queue.
  - narrow trailing sub-blocks are interleave-paired with a full one so
    PSUM h-buffer recycling never waits on the tanh pipeline.
"""

import numpy as np
import ml_dtypes

import concourse.bass as bass
import concourse.mybir as mybir
import concourse.tile as tile
from concourse import bacc
from concourse.bass import ts
from concourse.bass_utils import run_bass_kernel_spmd
BF16 = mybir.dt.bfloat16
FP32 = mybir.dt.float32
AFT = mybir.ActivationFunctionType

P = 128


class Cfg:
    def __init__(self, BL=4, NVS=(1152,) * 4, D=1024, H=4, F=512):
        # NVS: per-batch-slot padded valid-token count (batches are
        # assigned to slots sorted by count, so later slots can be shorter)
        self.BL, self.NVS, self.D, self.H, self.F = BL, tuple(NVS), D, H, F
        assert len(self.NVS) == BL
        self.NV = max(self.NVS)   # shipped tensor size
        self.HF = H * F
        self.KD = D // P          # k-chunks of D
        self.MC = self.HF // P    # f-chunks of H*F
        self.NCV = -(-self.NV // P)  # token chunks (max slot, ceil)
        self.KOUT = (H * D) // P  # k-chunks of the output projection

    def blocks(self, b):
        """DMA token blocks for batch slot b: 512-wide + remainder. A
        short (<=128) remainder is merged with 128 tokens borrowed from
        the previous block so the final chunk has two sub-blocks - the
        narrow one is interleave-paired with a full one to keep the PSUM
        buffer-recycle window wider than the tanh pipeline latency."""
        nv = self.NVS[b]
        out = [512] * (nv // 512)
        r = nv % 512
        if r:
            if r <= P and out:
                out[-1] -= P
                r += P
            out.append(r)
        return out


def choose_slots(valid_mask: np.ndarray, n_cores=8, BL=4):
    """Sort batches by valid count; slot i takes ranks [i*n_cores,
    (i+1)*n_cores) so each slot's NV covers its 8 batches exactly
    (partial trailing token chunks are fine - tokens ride free/K dims).
    Returns (NVS, order) with order[i*n_cores + c] = original batch
    index processed by core c in slot i."""
    cnt = np.asarray(valid_mask).sum(axis=1)
    order = np.argsort(-cnt, kind="stable")
    NVS = []
    for i in range(BL):
        grp = cnt[order[i * n_cores : (i + 1) * n_cores]]
        NVS.append(max(2, int(grp.max())))
    return tuple(NVS), order


def build_kernel(nc: bass.Bass, cfg: Cfg, reps: int = 1):
    c = cfg
    xt_d = nc.dram_tensor("xt", [c.BL, c.KD, P, c.NV], BF16, kind="ExternalInput").ap()
    xn_d = nc.dram_tensor("xn", [c.BL, c.NV, c.D], BF16, kind="ExternalInput").ap()
    v_d = nc.dram_tensor("v", [c.BL, P, c.NCV], BF16, kind="ExternalInput").ap()
    w1_d = nc.dram_tensor("w1", [c.KD, P, c.HF], BF16, kind="ExternalInput").ap()
    w2_d = nc.dram_tensor("w2", [c.MC, P, c.H], BF16, kind="ExternalInput").ap()
    b1_d = nc.dram_tensor("b1", [c.HF], FP32, kind="ExternalInput").ap()
    wout_d = nc.dram_tensor("wout", [c.KOUT, P, c.D], BF16, kind="ExternalInput").ap()
    bout_d = nc.dram_tensor("bout", [P, c.KD * c.BL], FP32, kind="ExternalInput").ap()
    y_d = nc.dram_tensor("y", [c.BL, c.D], FP32, kind="ExternalOutput").ap()

    with tile.TileContext(nc) as tc:
        with (
            tc.tile_pool(name="const", bufs=1) as const,
            tc.tile_pool(name="xT", bufs=3) as xT_pool,
            tc.tile_pool(name="h", bufs=4) as h_pool,
            tc.tile_pool(name="xn", bufs=3) as xn_pool,
            tc.tile_pool(name="e", bufs=6) as e_pool,
            tc.tile_pool(name="small", bufs=8) as small_pool,
            tc.tile_pool(name="ysb", bufs=1) as ysb_pool,
            tc.tile_pool(name="hps", bufs=4, space="PSUM") as hps_pool,
            tc.tile_pool(name="scps", bufs=2, space="PSUM") as scps_pool,
            tc.tile_pool(name="plps", bufs=2, space="PSUM") as plps_pool,
            tc.tile_pool(name="acc", bufs=2) as acc_pool,
        ):
            # ---- constants / weights ----
            # W1 as 4 column-quarter tiles: the first matmul group only
            # waits for quarter 0 (~1MB); the rest stream in behind it.
            QW = c.HF // 4
            QMC = c.MC // 4  # f-chunks per quarter
            w1q = [
                const.tile([P, c.KD, QW], BF16, tag=f"w1q{q}", name=f"w1q{q}")
                for q in range(4)
            ]

            def w1_piece(q, half, eng=None):
                # 256-col half-quarter: 512B innermost -> full DMA rate.
                # Halves alternate between the sync and vector DMA queues,
                # which transfer concurrently.
                f0 = q * QW + half * 256
                (eng or nc.sync).dma_start(
                    w1q[q][:, :, half * 256 : (half + 1) * 256],
                    w1_d[:, :, f0 : f0 + 256].rearrange("k p f -> p k f"),
                )

            w1_piece(0, 0)
            w2_sb = const.tile([P, c.MC, c.H], BF16)
            b1_sb = const.tile([P, c.MC], FP32)
            v_sb = const.tile([P, c.BL, c.NCV], BF16)
            bout_sb = const.tile([P, c.KD * c.BL], FP32)
            # pooled rows at r = b*32 + h (32-aligned per batch for DVE)
            # pooled^T accumulates directly in [128 d, (dc, h)] tiles; the
            # out-projection picks per-(dc,h) batch columns from poolT_sb
            poolT_sb = const.tile([P, c.KD, c.BL, c.H], BF16)
            ones_f32 = const.tile([1, P], FP32)
            nc.gpsimd.memset(ones_f32[:], 1.0)
            wout_sb = const.tile([P, c.KOUT, c.D], BF16)

            pending = []  # deferred (other-block) op emitters

            def flush(limit=0):
                # keep `limit` closures queued: pool matmuls lag their
                # sub-block by a few positions so early ones never stall
                # the PE on the xn DMA
                while len(pending) > limit:
                    pending.pop(0)()

            # flat chunk schedule across batches; xT DMAs prefetch 2 ahead
            sched = []
            for b in range(c.BL):
                t0 = 0
                for bi, TB in enumerate(c.blocks(b)):
                    sched.append((b, bi, t0, TB))
                    t0 += TB
            xts = {}

            def ensure_xt(k):
                if k >= len(sched) or k in xts:
                    return
                b, bi, t0, TB = sched[k]
                xT = xT_pool.tile([P, c.KD, 512], BF16, name=f"xT{k % 3}")
                if k == 0:
                    # first xT (half, if splittable) on the scalar queue:
                    # transfers concurrently with w1q0's first half on sync
                    h1 = 256 if TB == 512 else TB
                    nc.scalar.dma_start(
                        xT[:, :, 0:h1],
                        xt_d[b, :, :, 0:h1].rearrange("k p t -> p k t"),
                    )
                    # small weights behind it on the scalar queue
                    nc.scalar.dma_start(
                        b1_sb[:], b1_d.rearrange("(c p) -> p c", p=P)
                    )
                    nc.scalar.dma_start(
                        w2_sb[:], w2_d.rearrange("c p h -> p c h")
                    )
                    nc.scalar.dma_start(
                        v_sb[:], v_d.rearrange("b p c -> p b c")
                    )
                    nc.scalar.dma_start(bout_sb[:], bout_d)
                    # rest of W1 streams on sync + gpsimd concurrently
                    w1_piece(0, 1, nc.gpsimd)
                    for q in range(1, 4):
                        w1_piece(q, 0)
                        w1_piece(q, 1, nc.gpsimd)
                    if h1 < TB:
                        nc.sync.dma_start(
                            xT[:, :, h1:TB],
                            xt_d[b, :, :, h1:TB].rearrange("k p t -> p k t"),
                        )
                else:
                    nc.sync.dma_start(
                        xT[:, :, 0:TB],
                        xt_d[b, :, :, t0 : t0 + TB].rearrange("k p t -> p k t"),
                    )
                xts[k] = xT

            kprev = -1
            for b in range(c.BL):
                # pl/z PSUM tiles are bufs=1: allocate only after the
                # previous batch's deferred consumers have been emitted
                # (first flush of this batch), so buffer-reuse tracking
                # sees ops in order.
                z_ps = acc_sb = None
                NCVb = -(-c.NVS[b] // P)
                for bi, TB in enumerate(c.blocks(b)):
                    kprev += 1
                    k = kprev
                    _, _, t0, _ = sched[k]
                    cn0 = t0 // P
                    first = k == 0 and TB == 512
                    ensure_xt(k)
                    ensure_xt(k + 1)
                    ensure_xt(k + 2)
                    xT = xts.pop(k)
                    # xn rides the (otherwise idle) gpsimd DMA queue so it
                    # never contends with the xt/wout stream on sync
                    xnt = xn_pool.tile([P, 4, c.D], BF16)
                    fullt = (TB // P) * P  # whole 128-token chunks
                    if fullt:
                        nc.gpsimd.dma_start(
                            xnt[:, 0 : TB // P, :],
                            xn_d[b, t0 : t0 + fullt, :].rearrange(
                                "(s p) d -> p s d", p=P
                            ),
                        )
                    if TB % P:
                        nc.gpsimd.dma_start(
                            xnt[0 : TB % P, TB // P, :],
                            xn_d[b, t0 + fullt : t0 + TB, :],
                        )
                    if b == 1 and bi == 0:
                        # prefetch the output projection during the middle
                        KQ = c.KOUT // 4
                        for q in range(4):
                            nc.sync.dma_start(
                                wout_sb[:, ts(q, KQ), :],
                                wout_d[ts(q, KQ)].rearrange("k p f -> p k f"),
                            )
                    # 128-token compute sub-blocks; the first chunk of the
                    # kernel runs subs 0+1 interleaved so the W1 stream
                    # keeps up with the PE's f-chunk consumption.
                    subs = -(-TB // P)
                    wid = [min(P, TB - s * P) for s in range(subs)]
                    groups = [[s] for s in range(subs)]
                    if first:
                        groups = [[0, 1], [2], [3]]
                    elif bi == len(c.blocks(b)) - 1 and subs >= 2 and wid[-1] < P:
                        groups = groups[:-2] + [[subs - 2, subs - 1]]
                    for group in groups:
                        scs, prevq = {}, {}
                        for s in group:
                            scs[s] = scps_pool.tile(
                                [P, 512], FP32, tag="sc_ps", name=f"sc{s}"
                            )
                            prevq[s] = []

                        def dot(s, mc, h_sb):
                            nc.tensor.matmul(
                                scs[s][0 : wid[s], 0 : c.H],
                                h_sb[:, 0 : wid[s]],
                                w2_sb[:, mc, :],
                                start=(mc == 0),
                                stop=(mc == c.MC - 1),
                            )

                        for mc in range(c.MC):
                            for s in group:
                                w = wid[s]
                                h_ps = hps_pool.tile([P, 512], FP32, tag="h_ps")
                                wcol = (mc % QMC) * P
                                for dc in range(c.KD):
                                    nc.tensor.matmul(
                                        h_ps[:, 0:w],
                                        w1q[mc // QMC][:, dc, wcol : wcol + P],
                                        xT[:, dc, s * P : s * P + w],
                                        start=(dc == 0),
                                        stop=(dc == c.KD - 1),
                                    )
                                h_sb = h_pool.tile([P, P], BF16, tag="h_sb")
                                nc.scalar.activation(
                                    h_sb[:, 0:w], h_ps[:, 0:w], AFT.Tanh,
                                    bias=b1_sb[:, mc : mc + 1],
                                )
                                prevq[s].append((mc, h_sb))
                                if len(prevq[s]) > 2:
                                    dot(s, *prevq[s].pop(0))
                            if mc == 2:
                                flush(3)
                        for s in group:
                            while prevq[s]:
                                dot(s, *prevq[s].pop(0))
                        if acc_sb is None:
                            # full flush: the previous batch's pool/finish
                            # closures must be emitted before the acc
                            # buffer rotates to this batch
                            flush()
                            acc_sb = acc_pool.tile(
                                [P, c.KD * c.H + c.H], FP32, tag="acc"
                            )
                        for s in group:
                            w = wid[s]
                            e_blk = e_pool.tile([P, c.H], BF16, tag="e_blk")
                            nc.scalar.activation(
                                e_blk[0:w, :], scs[s][0:w, 0 : c.H],
                                AFT.Exp, bias=0.0,
                            )

                            def mk_pool(b, s, w, cn, ncv, e_blk, xnt, acc_sb):
                                def go():
                                    # pooled^T chunk: [128 d, H] per dc -
                                    # free dim 4, nearly free on the PE -
                                    # plus Z[h] = v . e on partition 0;
                                    # DVE accumulates chunks across cn
                                    NA = c.KD * c.H
                                    plw = plps_pool.tile(
                                        [P, 512], FP32, tag="plw", name="plw"
                                    )
                                    for dc in range(c.KD):
                                        nc.tensor.matmul(
                                            plw[:, dc * c.H : (dc + 1) * c.H],
                                            xnt[0:w, s, ts(dc, P)],
                                            e_blk[0:w, :],
                                            start=True,
                                            stop=True,
                                        )
                                    nc.tensor.matmul(
                                        plw[0:1, NA : NA + c.H],
                                        v_sb[0:w, b, cn : cn + 1],
                                        e_blk[0:w, :],
                                        start=True,
                                        stop=True,
                                    )
                                    if cn == 0:
                                        nc.vector.tensor_copy(
                                            acc_sb[:, 0:NA], plw[:, 0:NA]
                                        )
                                        nc.vector.tensor_copy(
                                            acc_sb[0:1, NA : NA + c.H],
                                            plw[0:1, NA : NA + c.H],
                                        )
                                    else:
                                        nc.vector.tensor_add(
                                            acc_sb[:, 0:NA], acc_sb[:, 0:NA],
                                            plw[:, 0:NA],
                                        )
                                        nc.vector.tensor_add(
                                            acc_sb[0:1, NA : NA + c.H],
                                            acc_sb[0:1, NA : NA + c.H],
                                            plw[0:1, NA : NA + c.H],
                                        )
                                return go

                            pending.append(
                                mk_pool(b, s, w, cn0 + s, NCVb, e_blk, xnt,
                                        acc_sb)
                            )
                    t0 += TB

                def mk_finish(b, acc_sb):
                    def go():
                        NA = c.KD * c.H
                        rz = small_pool.tile([1, c.H], FP32, tag="rz")
                        nc.vector.reciprocal(rz[:], acc_sb[0:1, NA : NA + c.H])
                        # broadcast rz down the partitions via the PE, then
                        # scale acc into poolT_sb one head-column at a time
                        rzb_ps = scps_pool.tile(
                            [P, 512], FP32, tag="sc_ps", name="rzb"
                        )
                        nc.tensor.matmul(
                            rzb_ps[:, 0 : c.H], ones_f32[:], rz[:],
                            start=True, stop=True,
                        )
                        rzb = small_pool.tile([P, c.H], FP32, tag="rzb")
                        nc.vector.tensor_copy(rzb[:], rzb_ps[:, 0 : c.H])
                        accv = acc_sb[:, 0:NA].rearrange(
                            "p (k h) -> p h k", h=c.H
                        )
                        for hd in range(c.H):
                            nc.vector.tensor_scalar_mul(
                                poolT_sb[:, :, b, hd],
                                accv[:, hd, :],
                                rzb[:, hd : hd + 1],
                            )
                    return go

                pending.append(mk_finish(b, acc_sb))

            flush()
            # ---- output projection y = pooled @ Wout + bout ----
            # y^T chunks [128 d_out, BL]: free dim = BL = 4, so the 256
            # matmuls cost the PE almost nothing (vs 64 x 512-free rows
            # the straight orientation would charge). Groups sequential
            # per d_out chunk -> one PSUM region suffices.
            yt_ps = plps_pool.tile([P, 512], FP32, tag="plw", name="yt_ps")
            ytcols = yt_ps[:, 0 : c.BL * c.KD].rearrange(
                "p (b o) -> p o b", o=c.KD
            )
            for do in range(c.KD):
                for hd in range(c.H):
                    for dc in range(c.KD):
                        nc.tensor.matmul(
                            ytcols[:, do, :],
                            wout_sb[:, hd * c.KD + dc, ts(do, P)],
                            poolT_sb[:, dc, :, hd],
                            start=(hd == 0 and dc == 0),
                            stop=(hd == c.H - 1 and dc == c.KD - 1),
                        )
            # += bout (pre-laid as [P, KD*BL]), then a strided DMA writes
            # y directly in [BL, D] order - no transposes, no extra copy
            yt_sb = ysb_pool.tile([P, c.KD * c.BL], FP32)
            nc.vector.tensor_add(
                yt_sb[:], yt_ps[:, 0 : c.KD * c.BL], bout_sb[:]
            )
            nc.sync.dma_start(
                y_d.rearrange("b (o p) -> p b o", p=P),
                yt_sb[:].rearrange("p (b o) -> p b o", o=c.KD),
            )
    return nc


def make_in_maps(x, valid_mask, W1, b1, W2, b2, Wout, bout, n_cores, cfg,
                 order=None):
    """Host-side prep: compact valid tokens, shard over batch, cast/layout."""
    c = cfg
    bf16 = ml_dtypes.bfloat16
    B = x.shape[0]
    w1_l = np.ascontiguousarray(
        W1.transpose(1, 0, 2).reshape(c.KD, P, c.HF).astype(bf16)
    )
    w2f = W2.reshape(c.HF).astype(np.float32)
    w2_l = np.zeros((c.MC, P, c.H), np.float32)
    FC = c.MC // c.H  # f-chunks per head
    for mc in range(c.MC):
        w2_l[mc, :, mc // FC] = w2f[mc * P : (mc + 1) * P]
    w2_l = np.ascontiguousarray(w2_l.astype(bf16))
    b1_l = np.ascontiguousarray(b1.reshape(c.HF).astype(np.float32))
    wout_l = np.ascontiguousarray(Wout.reshape(c.KOUT, P, c.D).astype(bf16))
    # bout pre-laid for the y^T tail: bout_l[p, b*KD + do] = bout[do*128+p]
    bout_l = np.ascontiguousarray(
        np.tile(bout.astype(np.float32).reshape(c.KD, P).T, (1, c.BL))
    )
    # b2 is a per-row constant under the softmax -> it cancels; drop it.
    if order is None:
        order = np.arange(B)
    xc = np.zeros((B, c.NV, c.D), bf16)
    v = np.zeros((B, c.NCV * P), np.float32)
    for gb in range(B):
        idx = np.flatnonzero(valid_mask[gb])[: c.NV]
        xc[gb, : len(idx)] = x[gb, idx].astype(bf16)
        v[gb, : len(idx)] = 1.0
    v_l = np.ascontiguousarray(
        v.reshape(B, c.NCV, P).transpose(0, 2, 1).astype(bf16)
    )
    xt_all = np.ascontiguousarray(xc.transpose(0, 2, 1)).reshape(
        B, c.KD, P, c.NV
    )
    in_maps = []
    for core in range(n_cores):
        sel = [order[i * n_cores + core] for i in range(c.BL)]
        in_maps.append(
            {
                "xt": np.ascontiguousarray(xt_all[sel]),
                "xn": np.ascontiguousarray(xc[sel]),
                "v": np.ascontiguousarray(v_l[sel]),
                "w1": w1_l,
                "w2": w2_l,
                "b1": b1_l,
                "wout": wout_l,
                "bout": bout_l,
            }
        )
    return in_maps


_cached = {}
last_results = None


def kernel(x, valid_mask, W1, b1, W2, b2, Wout, bout, trace=False):
    global last_results
    x, valid_mask, W1, b1, W2, b2, Wout, bout = (
        np.asarray(a)
        for a in (x, valid_mask, W1, b1, W2, b2, Wout, bout)
    )
    B = x.shape[0]
    n_cores = 8
    NVS, order = choose_slots(valid_mask, n_cores, B // n_cores)
    cfg = Cfg(BL=B // n_cores, NVS=NVS)
    key = (B, NVS)
    if key not in _cached:
        nc = bacc.Bacc("TRN2", target_bir_lowering=False, debug=False)
        build_kernel(nc, cfg)
        nc.compile()
        _cached[key] = nc
    in_maps = make_in_maps(
        x, valid_mask, W1, b1, W2, b2, Wout, bout, n_cores, cfg, order
    )
    res = run_bass_kernel_spmd(
        _cached[key], in_maps, core_ids=list(range(n_cores)), trace=trace
    )
    last_results = res
    y = np.empty((B, x.shape[2]), np.float32)
    for core in range(n_cores):
        yc = np.asarray(res.results[core]["y"], np.float32)
        for i in range(cfg.BL):
            y[order[i * n_cores + core]] = yc[i]
    return y


# revision 72
# speedup vs baseline: 1.0008x; 1.0008x over previous
"""AttentionPool Trainium2 Bass kernel (valid-token compaction).

Reference computation (per batch b):
    h      = tanh(x @ W1 + b1)          # [N, H*F]   (big matmul, bf16 on PE)
    scores = h @ W2 + b2                # [N, H]
    scores = where(mask, scores, -1e9)
    w      = softmax(scores, axis=N)    # per head
    pooled = w.T @ x                    # [H, D]
    y      = concat_h(pooled) @ Wout + bout   # [D]

Key structural ideas:
  - Invalid tokens get softmax weight 0 and contribute nothing to the
    output, so the host compacts each batch's valid tokens (~1024 of 2048
    at p=0.5) into a contiguous buffer and the big x@W1 matmul runs only
    on those (~2x less PE work). Batches are sorted by valid count and
    assigned to per-slot lengths NVS (exact, partial trailing 128-token
    chunks are fine), so every core does the same near-minimal work.
  - Padding inside a slot carries x=0: it cannot pollute the pooling sum,
    and the softmax denominator is computed as a validity-weighted PE
    reduction Z = sum_t v_t e_t (v in {1,0}), so no -1e9 mask tensor or
    max-shift exists anywhere. b2 cancels under softmax and is dropped.
  - All small-output reductions are shaped so the PE streams tiny free
    dims (matmul cost ~ out free size): the score dot emits [tok, H=4]
    tiles, pooling emits pooled^T [128d, H=4] tiles per (chunk, dc) that
    the DVE accumulates across chunks, and the output projection emits
    y^T [128d, B=4] columns. Everything lands pre-transposed for its
    consumer; the only PE transposes left are none.
  - The exp(scores) tiles are already token-major = the pooling lhsT.

Sharding: data-parallel over batch B=32 across 8 cores (4 batches/core,
count-sorted slot assignment, results un-permuted on the host). Weights
replicated. Matmuls bf16 (fp32 PSUM); softmax fp32 on the Act engine
(the exp_and_others act table holds both tanh and exp - no table loads).
|scores| <= ||W2||_1 ~ 18, so exp is safe without max-shift.

Pipelining (keeps the PE dense, which also keeps its p-state ramped):
  - per f-chunk mc: the 8 k-matmuls of chunk mc+1..mc+2 are issued before
    the score dot of chunk mc (depth-2 queue), hiding the tanh latency.
  - a sub-block's pooling/Z matmuls are deferred 3 sub-blocks down the
    instruction stream, when exp and the xn DMA have long finished.
  - W1 streams in 256-column half-quarters on two DMA queues (sync +
    gpsimd) paced to the first chunk's interleaved sub-pair consumption;
    xT prefetches 2 chunks ahead; xn rides the gpsimd queue.
  - narrow trailing sub-blocks are interleave-paired with a full one so
    PSUM h-buffer recycling never waits on the tanh pipeline.
"""

import numpy as np
import ml_dtypes

import concourse.bass as bass
import concourse.mybir as mybir
import concourse.tile as tile
from concourse import bacc
from concourse.bass import ts
from concourse.bass_utils import run_bass_kernel_spmd
BF16 = mybir.dt.bfloat16
FP32 = mybir.dt.float32
AFT = mybir.ActivationFunctionType

P = 128


class Cfg:
    def __init__(self, BL=4, NVS=(1152,) * 4, D=1024, H=4, F=512):
        # NVS: per-batch-slot padded valid-token count (batches are
        # assigned to slots sorted by count, so later slots can be shorter)
        self.BL, self.NVS, self.D, self.H, self.F = BL, tuple(NVS), D, H, F
        assert len(self.NVS) == BL
        self.NV = max(self.NVS)   # shipped tensor size
        self.HF = H * F
        self.KD = D // P          # k-chunks of D
        self.MC = self.HF // P    # f-chunks of H*F
        self.NCV = -(-self.NV // P)  # token chunks (max slot, ceil)
        self.KOUT = (H * D) // P  # k-chunks of the output projection

    def blocks(self, b):
        """DMA token blocks for batch slot b: 512-wide + remainder. A
        short (<=128) remainder is merged with 128 tokens borrowed from
        the previous block so the final chunk has two sub-blocks - the
        narrow one is interleave-paired with a full one to keep the PSUM
        buffer-recycle window wider than the tanh pipeline latency."""
        nv = self.NVS[b]
        out = [512] * (nv // 512)
        r = nv % 512
        if r:
            if r <= P and out:
                out[-1] -= P
                r += P
            out.append(r)
        return out


def choose_slots(valid_mask: np.ndarray, n_cores=8, BL=4):
    """Sort batches by valid count; slot i takes ranks [i*n_cores,
    (i+1)*n_cores) so each slot's NV covers its 8 batches exactly
    (partial trailing token chunks are fine - tokens ride free/K dims).
    Returns (NVS, order) with order[i*n_cores + c] = original batch
    index processed by core c in slot i."""
    cnt = np.asarray(valid_mask).sum(axis=1)
    order = np.argsort(-cnt, kind="stable")
    NVS = []
    for i in range(BL):
        grp = cnt[order[i * n_cores : (i + 1) * n_cores]]
        NVS.append(max(2, int(grp.max())))
    return tuple(NVS), order


def build_kernel(nc: bass.Bass, cfg: Cfg, reps: int = 1):
    c = cfg
    xt_d = nc.dram_tensor("xt", [c.BL, c.KD, P, c.NV], BF16, kind="ExternalInput").ap()
    xn_d = nc.dram_tensor("xn", [c.BL, c.NV, c.D], BF16, kind="ExternalInput").ap()
    v_d = nc.dram_tensor("v", [c.BL, P, c.NCV], BF16, kind="ExternalInput").ap()
    w1_d = nc.dram_tensor("w1", [c.KD, P, c.HF], BF16, kind="ExternalInput").ap()
    w2_d = nc.dram_tensor("w2", [c.MC, P, c.H], BF16, kind="ExternalInput").ap()
    b1_d = nc.dram_tensor("b1", [c.HF], FP32, kind="ExternalInput").ap()
    wout_d = nc.dram_tensor("wout", [c.KOUT, P, c.D], BF16, kind="ExternalInput").ap()
    bout_d = nc.dram_tensor("bout", [P, c.KD * c.BL], FP32, kind="ExternalInput").ap()
    y_d = nc.dram_tensor("y", [c.BL, c.D], FP32, kind="ExternalOutput").ap()

    with tile.TileContext(nc) as tc:
        with (
            tc.tile_pool(name="const", bufs=1) as const,
            tc.tile_pool(name="xT", bufs=3) as xT_pool,
            tc.tile_pool(name="h", bufs=4) as h_pool,
            tc.tile_pool(name="xn", bufs=3) as xn_pool,
            tc.tile_pool(name="e", bufs=6) as e_pool,
            tc.tile_pool(name="small", bufs=8) as small_pool,
            tc.tile_pool(name="ysb", bufs=1) as ysb_pool,
            tc.tile_pool(name="hps", bufs=4, space="PSUM") as hps_pool,
            tc.tile_pool(name="scps", bufs=2, space="PSUM") as scps_pool,
            tc.tile_pool(name="plps", bufs=2, space="PSUM") as plps_pool,
            tc.tile_pool(name="acc", bufs=2) as acc_pool,
        ):
            # ---- constants / weights ----
            # W1 as 4 column-quarter tiles: the first matmul group only
            # waits for quarter 0 (~1MB); the rest stream in behind it.
            QW = c.HF // 4
            QMC = c.MC // 4  # f-chunks per quarter
            w1q = [
                const.tile([P, c.KD, QW], BF16, tag=f"w1q{q}", name=f"w1q{q}")
                for q in range(4)
            ]

            def w1_piece(q, half, eng=None):
                # 256-col half-quarter: 512B innermost -> full DMA rate.
                # Halves alternate between the sync and vector DMA queues,
                # which transfer concurrently.
                f0 = q * QW + half * 256
                (eng or nc.sync).dma_start(
                    w1q[q][:, :, half * 256 : (half + 1) * 256],
                    w1_d[:, :, f0 : f0 + 256].rearrange("k p f -> p k f"),
                )

            w1_piece(0, 0)
            w2_sb = const.tile([P, c.MC, c.H], BF16)
            b1_sb = const.tile([P, c.MC], FP32)
            v_sb = const.tile([P, c.BL, c.NCV], BF16)
            bout_sb = const.tile([P, c.KD * c.BL], FP32)
            # pooled rows at r = b*32 + h (32-aligned per batch for DVE)
            # pooled^T accumulates directly in [128 d, (dc, h)] tiles; the
            # out-projection picks per-(dc,h) batch columns from poolT_sb
            poolT_sb = const.tile([P, c.KD, c.BL, c.H], BF16)
            ones_f32 = const.tile([1, P], FP32)
            nc.gpsimd.memset(ones_f32[:], 1.0)
            wout_sb = const.tile([P, c.KOUT, c.D], BF16)

            pending = []  # deferred (other-block) op emitters

            def flush(limit=0):
                # keep `limit` closures queued: pool matmuls lag their
                # sub-block by a few positions so early ones never stall
                # the PE on the xn DMA
                while len(pending) > limit:
                    pending.pop(0)()

            # flat chunk schedule across batches; xT DMAs prefetch 2 ahead
            sched = []
            for b in range(c.BL):
                t0 = 0
                for bi, TB in enumerate(c.blocks(b)):
                    sched.append((b, bi, t0, TB))
                    t0 += TB
            xts = {}

            def ensure_xt(k):
                if k >= len(sched) or k in xts:
                    return
                b, bi, t0, TB = sched[k]
                xT = xT_pool.tile([P, c.KD, 512], BF16, name=f"xT{k % 3}")
                if k == 0:
                    # first xT (half, if splittable) on the scalar queue:
                    # transfers concurrently with w1q0's first half on sync
                    h1 = 256 if TB == 512 else TB
                    nc.scalar.dma_start(
                        xT[:, :, 0:h1],
                        xt_d[b, :, :, 0:h1].rearrange("k p t -> p k t"),
                    )
                    # small weights behind it on the scalar queue
                    nc.scalar.dma_start(
                        b1_sb[:], b1_d.rearrange("(c p) -> p c", p=P)
                    )
                    nc.scalar.dma_start(
                        w2_sb[:], w2_d.rearrange("c p h -> p c h")
                    )
                    nc.scalar.dma_start(
                        v_sb[:], v_d.rearrange("b p c -> p b c")
                    )
                    nc.scalar.dma_start(bout_sb[:], bout_d)
                    # rest of W1 streams on sync + gpsimd concurrently
                    w1_piece(0, 1, nc.gpsimd)
                    for q in range(1, 4):
                        w1_piece(q, 0)
                        w1_piece(q, 1, nc.gpsimd)
                    if h1 < TB:
                        nc.sync.dma_start(
                            xT[:, :, h1:TB],
                            xt_d[b, :, :, h1:TB].rearrange("k p t -> p k t"),
                        )
                else:
                    nc.sync.dma_start(
                        xT[:, :, 0:TB],
                        xt_d[b, :, :, t0 : t0 + TB].rearrange("k p t -> p k t"),
                    )
                xts[k] = xT

            kprev = -1
            for b in range(c.BL):
                # pl/z PSUM tiles are bufs=1: allocate only after the
                # previous batch's deferred consumers have been emitted
                # (first flush of this batch), so buffer-reuse tracking
                # sees ops in order.
                z_ps = acc_sb = None
                NCVb = -(-c.NVS[b] // P)
                for bi, TB in enumerate(c.blocks(b)):
                    kprev += 1
                    k = kprev
                    _, _, t0, _ = sched[k]
                    cn0 = t0 // P
                    first = k == 0 and TB == 512
                    ensure_xt(k)
                    ensure_xt(k + 1)
                    ensure_xt(k + 2)
                    xT = xts.pop(k)
                    # xn rides the (otherwise idle) gpsimd DMA queue so it
                    # never contends with the xt/wout stream on sync
                    xnt = xn_pool.tile([P, 4, c.D], BF16)
                    fullt = (TB // P) * P  # whole 128-token chunks
                    if fullt:
                        nc.gpsimd.dma_start(
                            xnt[:, 0 : TB // P, :],
                            xn_d[b, t0 : t0 + fullt, :].rearrange(
                                "(s p) d -> p s d", p=P
                            ),
                        )
                    if TB % P:
                        nc.gpsimd.dma_start(
                            xnt[0 : TB % P, TB // P, :],
                            xn_d[b, t0 + fullt : t0 + TB, :],
                        )
                    if b == 1 and bi == 0:
                        # prefetch the output projection during the middle
                        KQ = c.KOUT // 4
                        for q in range(4):
                            nc.sync.dma_start(
                                wout_sb[:, ts(q, KQ), :],
                                wout_d[ts(q, KQ)].rearrange("k p f -> p k f"),
                            )
                    # 128-token compute sub-blocks; the first chunk of the
                    # kernel runs subs 0+1 interleaved so the W1 stream
                    # keeps up with the PE's f-chunk consumption.
                    subs = -(-TB // P)
                    wid = [min(P, TB - s * P) for s in range(subs)]
                    groups = [[s] for s in range(subs)]
                    if first:
                        groups = [[0, 1], [2], [3]]
                    elif bi == len(c.blocks(b)) - 1 and subs >= 2 and wid[-1] < P:
                        groups = groups[:-2] + [[subs - 2, subs - 1]]
                    for group in groups:
                        scs, prevq = {}, {}
                        for s in group:
                            scs[s] = scps_pool.tile(
                                [P, 512], FP32, tag="sc_ps", name=f"sc{s}"
                            )
                            prevq[s] = []

                        def dot(s, mc, h_sb):
                            nc.tensor.matmul(
                                scs[s][0 : wid[s], 0 : c.H],
                                h_sb[:, 0 : wid[s]],
                                w2_sb[:, mc, :],
                                start=(mc == 0),
                                stop=(mc == c.MC - 1),
                            )

                        for mc in range(c.MC):
                            for s in group:
                                w = wid[s]
                                h_ps = hps_pool.tile([P, 512], FP32, tag="h_ps")
                                wcol = (mc % QMC) * P
                                for dc in range(c.KD):
                                    nc.tensor.matmul(
                                        h_ps[:, 0:w],
                                        w1q[mc // QMC][:, dc, wcol : wcol + P],
                                        xT[:, dc, s * P : s * P + w],
                                        start=(dc == 0),
                                        stop=(dc == c.KD - 1),
                                    )
                                h_sb = h_pool.tile([P, P], BF16, tag="h_sb")
                                nc.scalar.activation(
                                    h_sb[:, 0:w], h_ps[:, 0:w], AFT.Tanh,
                                    bias=b1_sb[:, mc : mc + 1],
                                )
                                prevq[s].append((mc, h_sb))
                                if len(prevq[s]) > 2:
                                    dot(s, *prevq[s].pop(0))
                            if mc == 2:
                                # drain fully in the kernel's last chunk so
                                # deferred pool work doesn't pile into the tail
                                flush(0 if k == len(sched) - 1 else 3)
                        for s in group:
                            while prevq[s]:
                                dot(s, *prevq[s].pop(0))
                        if acc_sb is None:
                            # full flush: the previous batch's pool/finish
                            # closures must be emitted before the acc
                            # buffer rotates to this batch
                            flush()
                            acc_sb = acc_pool.tile(
                                [P, c.KD * c.H + c.H], FP32, tag="acc"
                            )
                        for s in group:
                            w = wid[s]
                            e_blk = e_pool.tile([P, c.H], BF16, tag="e_blk")
                            nc.scalar.activation(
                                e_blk[0:w, :], scs[s][0:w, 0 : c.H],
                                AFT.Exp, bias=0.0,
                            )

                            def mk_pool(b, s, w, cn, ncv, e_blk, xnt, acc_sb):
                                def go():
                                    # pooled^T chunk: [128 d, H] per dc -
                                    # free dim 4, nearly free on the PE -
                                    # plus Z[h] = v . e on partition 0;
                                    # DVE accumulates chunks across cn
                                    NA = c.KD * c.H
                                    plw = plps_pool.tile(
                                        [P, 512], FP32, tag="plw", name="plw"
                                    )
                                    for dc in range(c.KD):
                                        nc.tensor.matmul(
                                            plw[:, dc * c.H : (dc + 1) * c.H],
                                            xnt[0:w, s, ts(dc, P)],
                                            e_blk[0:w, :],
                                            start=True,
                                            stop=True,
                                        )
                                    nc.tensor.matmul(
                                        plw[0:1, NA : NA + c.H],
                                        v_sb[0:w, b, cn : cn + 1],
                                        e_blk[0:w, :],
                                        start=True,
                                        stop=True,
                                    )
                                    if cn == 0:
                                        nc.vector.tensor_copy(
                                            acc_sb[:, 0:NA], plw[:, 0:NA]
                                        )
                                        nc.vector.tensor_copy(
                                            acc_sb[0:1, NA : NA + c.H],
                                            plw[0:1, NA : NA + c.H],
                                        )
                                    else:
                                        nc.vector.tensor_add(
                                            acc_sb[:, 0:NA], acc_sb[:, 0:NA],
                                            plw[:, 0:NA],
                                        )
                                        nc.vector.tensor_add(
                                            acc_sb[0:1, NA : NA + c.H],
                                            acc_sb[0:1, NA : NA + c.H],
                                            plw[0:1, NA : NA + c.H],
                                        )
                                return go

                            pending.append(
                                mk_pool(b, s, w, cn0 + s, NCVb, e_blk, xnt,
                                        acc_sb)
                            )
                    t0 += TB

                def mk_finish(b, acc_sb):
                    def go():
                        NA = c.KD * c.H
                        rz = small_pool.tile([1, c.H], FP32, tag="rz")
                        nc.vector.reciprocal(rz[:], acc_sb[0:1, NA : NA + c.H])
                        # broadcast rz down the partitions via the PE, then
                        # scale acc into poolT_sb one head-column at a time
                        rzb_ps = scps_pool.tile(
                            [P, 512], FP32, tag="sc_ps", name="rzb"
                        )
                        nc.tensor.matmul(
                            rzb_ps[:, 0 : c.H], ones_f32[:], rz[:],
                            start=True, stop=True,
                        )
                        rzb = small_pool.tile([P, c.H], FP32, tag="rzb")
                        nc.vector.tensor_copy(rzb[:], rzb_ps[:, 0 : c.H])
                        accv = acc_sb[:, 0:NA].rearrange(
                            "p (k h) -> p h k", h=c.H
                        )
                        for hd in range(c.H):
                            nc.vector.tensor_scalar_mul(
                                poolT_sb[:, :, b, hd],
                                accv[:, hd, :],
                                rzb[:, hd : hd + 1],
                            )
                    return go

                pending.append(mk_finish(b, acc_sb))

            flush()
            # ---- output projection y = pooled @ Wout + bout ----
            # y^T chunks [128 d_out, BL]: free dim = BL = 4, so the 256
            # matmuls cost the PE almost nothing (vs 64 x 512-free rows
            # the straight orientation would charge). Groups sequential
            # per d_out chunk -> one PSUM region suffices.
            yt_ps = plps_pool.tile([P, 512], FP32, tag="plw", name="yt_ps")
            ytcols = yt_ps[:, 0 : c.BL * c.KD].rearrange(
                "p (b o) -> p o b", o=c.KD
            )
            for do in range(c.KD):
                for hd in range(c.H):
                    for dc in range(c.KD):
                        nc.tensor.matmul(
                            ytcols[:, do, :],
                            wout_sb[:, hd * c.KD + dc, ts(do, P)],
                            poolT_sb[:, dc, :, hd],
                            start=(hd == 0 and dc == 0),
                            stop=(hd == c.H - 1 and dc == c.KD - 1),
                        )
            # += bout (pre-laid as [P, KD*BL]), then a strided DMA writes
            # y directly in [BL, D] order - no transposes, no extra copy
            yt_sb = ysb_pool.tile([P, c.KD * c.BL], FP32)
            nc.vector.tensor_add(
                yt_sb[:], yt_ps[:, 0 : c.KD * c.BL], bout_sb[:]
            )
            nc.sync.dma_start(
                y_d.rearrange("b (o p) -> p b o", p=P),
                yt_sb[:].rearrange("p (b o) -> p b o", o=c.KD),
            )
    return nc


def make_in_maps(x, valid_mask, W1, b1, W2, b2, Wout, bout, n_cores, cfg,
                 order=None):
    """Host-side prep: compact valid tokens, shard over batch, cast/layout."""
    c = cfg
    bf16 = ml_dtypes.bfloat16
    B = x.shape[0]
    w1_l = np.ascontiguousarray(
        W1.transpose(1, 0, 2).reshape(c.KD, P, c.HF).astype(bf16)
    )
    w2f = W2.reshape(c.HF).astype(np.float32)
    w2_l = np.zeros((c.MC, P, c.H), np.float32)
    FC = c.MC // c.H  # f-chunks per head
    for mc in range(c.MC):
        w2_l[mc, :, mc // FC] = w2f[mc * P : (mc + 1) * P]
    w2_l = np.ascontiguousarray(w2_l.astype(bf16))
    b1_l = np.ascontiguousarray(b1.reshape(c.HF).astype(np.float32))
    wout_l = np.ascontiguousarray(Wout.reshape(c.KOUT, P, c.D).astype(bf16))
    # bout pre-laid for the y^T tail: bout_l[p, b*KD + do] = bout[do*128+p]
    bout_l = np.ascontiguousarray(
        np.tile(bout.astype(np.float32).reshape(c.KD, P).T, (1, c.BL))
    )
    # b2 is a per-row constant under the softmax -> it cancels; drop it.
    if order is None:
        order = np.arange(B)
    xc = np.zeros((B, c.NV, c.D), bf16)
    v = np.zeros((B, c.NCV * P), np.float32)
    for gb in range(B):
        idx = np.flatnonzero(valid_mask[gb])[: c.NV]
        xc[gb, : len(idx)] = x[gb, idx].astype(bf16)
        v[gb, : len(idx)] = 1.0
    v_l = np.ascontiguousarray(
        v.reshape(B, c.NCV, P).transpose(0, 2, 1).astype(bf16)
    )
    xt_all = np.ascontiguousarray(xc.transpose(0, 2, 1)).reshape(
        B, c.KD, P, c.NV
    )
    in_maps = []
    for core in range(n_cores):
        sel = [order[i * n_cores + core] for i in range(c.BL)]
        in_maps.append(
            {
                "xt": np.ascontiguousarray(xt_all[sel]),
                "xn": np.ascontiguousarray(xc[sel]),
                "v": np.ascontiguousarray(v_l[sel]),
                "w1": w1_l,
                "w2": w2_l,
                "b1": b1_l,
                "wout": wout_l,
                "bout": bout_l,
            }
        )
    return in_maps


_cached = {}
last_results = None


def kernel(x, valid_mask, W1, b1, W2, b2, Wout, bout, trace=False):
    global last_results
    x, valid_mask, W1, b1, W2, b2, Wout, bout = (
        np.asarray(a)
        for a in (x, valid_mask, W1, b1, W2, b2, Wout, bout)
    )
    B = x.shape[0]
    n_cores = 8
    NVS, order = choose_slots(valid_mask, n_cores, B // n_cores)
    cfg = Cfg(BL=B // n_cores, NVS=NVS)
    key = (B, NVS)
    if key not in _cached:
        nc = bacc.Bacc("TRN2", target_bir_lowering=False, debug=False)
        build_kernel(nc, cfg)
        nc.compile()
        _cached[key] = nc
    in_maps = make_in_maps(
        x, valid_mask, W1, b1, W2, b2, Wout, bout, n_cores, cfg, order
    )
    res = run_bass_kernel_spmd(
        _cached[key], in_maps, core_ids=list(range(n_cores)), trace=trace
    )
    last_results = res
    y = np.empty((B, x.shape[2]), np.float32)
    for core in range(n_cores):
        yc = np.asarray(res.results[core]["y"], np.float32)
        for i in range(cfg.BL):
            y[order[i * n_cores + core]] = yc[i]
    return y


# revision 81
# speedup vs baseline: 1.2906x; 1.2896x over previous
"""AttentionPool Trainium2 Bass kernel (valid-token compaction).

Reference computation (per batch b):
    h      = tanh(x @ W1 + b1)          # [N, H*F]   (big matmul, bf16 on PE)
    scores = h @ W2 + b2                # [N, H]
    scores = where(mask, scores, -1e9)
    w      = softmax(scores, axis=N)    # per head
    pooled = w.T @ x                    # [H, D]
    y      = concat_h(pooled) @ Wout + bout   # [D]

Key structural ideas:
  - Invalid tokens get softmax weight 0 and contribute nothing to the
    output, so the host compacts each batch's valid tokens (~1024 of 2048
    at p=0.5) into a contiguous buffer and the big x@W1 matmul runs only
    on those (~2x less PE work). Batches are sorted by valid count and
    assigned to per-slot lengths NVS (exact, partial trailing 128-token
    chunks are fine), so every core does the same near-minimal work.
  - Padding inside a slot carries x=0: it cannot pollute the pooling sum,
    and the softmax denominator is computed as a validity-weighted PE
    reduction Z = sum_t v_t e_t (v in {1,0}), so no -1e9 mask tensor or
    max-shift exists anywhere. b2 cancels under softmax and is dropped.
  - All small-output reductions are shaped so the PE streams tiny free
    dims (matmul cost ~ out free size): the score dot emits [tok, H=4]
    tiles, pooling emits pooled^T [128d, H=4] tiles per (chunk, dc) that
    the DVE accumulates across chunks, and the output projection emits
    y^T [128d, B=4] columns. Everything lands pre-transposed for its
    consumer; the only PE transposes left are none.
  - The exp(scores) tiles are already token-major = the pooling lhsT.

Sharding: data-parallel over batch B=32 across 8 cores (4 batches/core,
count-sorted slot assignment, results un-permuted on the host). Weights
replicated. Matmuls bf16 (fp32 PSUM); softmax fp32 on the Act engine
(the exp_and_others act table holds both tanh and exp - no table loads).
|scores| <= ||W2||_1 ~ 18, so exp is safe without max-shift.

Pipelining (keeps the PE dense, which also keeps its p-state ramped):
  - per f-chunk mc: the 8 k-matmuls of chunk mc+1..mc+2 are issued before
    the score dot of chunk mc (depth-2 queue), hiding the tanh latency.
  - a sub-block's pooling/Z matmuls are deferred 3 sub-blocks down the
    instruction stream, when exp and the xn DMA have long finished.
  - W1 streams in 256-column half-quarters on two DMA queues (sync +
    gpsimd) paced to the first chunk's interleaved sub-pair consumption;
    xT prefetches 2 chunks ahead; xn rides the gpsimd queue.
  - narrow trailing sub-blocks are interleave-paired with a full one so
    PSUM h-buffer recycling never waits on the tanh pipeline.
"""

import numpy as np
import ml_dtypes

import concourse.bass as bass
import concourse.mybir as mybir
import concourse.tile as tile
from concourse import bacc
from concourse.bass import ts
from concourse.bass_utils import run_bass_kernel_spmd
BF16 = mybir.dt.bfloat16
FP32 = mybir.dt.float32
F8H = mybir.dt.float8e4
F8L = mybir.dt.float8e5
DR = mybir.MatmulPerfMode.DoubleRow
AFT = mybir.ActivationFunctionType

P = 128


class Cfg:
    def __init__(self, BL=4, NVS=(1152,) * 4, D=1024, H=4, F=512):
        # NVS: per-batch-slot padded valid-token count (batches are
        # assigned to slots sorted by count, so later slots can be shorter)
        self.BL, self.NVS, self.D, self.H, self.F = BL, tuple(NVS), D, H, F
        assert len(self.NVS) == BL
        self.NV = max(self.NVS)   # shipped tensor size
        self.HF = H * F
        self.KD = D // P          # k-chunks of D
        self.MC = self.HF // P    # f-chunks of H*F
        self.NCV = -(-self.NV // P)  # token chunks (max slot, ceil)
        self.KOUT = (H * D) // P  # k-chunks of the output projection

    def blocks(self, b):
        """DMA token blocks for batch slot b: 512-wide + remainder. A
        short (<=128) remainder is merged with 128 tokens borrowed from
        the previous block so the final chunk has two sub-blocks - the
        narrow one is interleave-paired with a full one to keep the PSUM
        buffer-recycle window wider than the tanh pipeline latency."""
        nv = self.NVS[b]
        out = [512] * (nv // 512)
        r = nv % 512
        if r:
            if r <= P and out:
                out[-1] -= P
                r += P
            out.append(r)
        return out


def choose_slots(valid_mask: np.ndarray, n_cores=8, BL=4):
    """Sort batches by valid count; slot i takes ranks [i*n_cores,
    (i+1)*n_cores) so each slot's NV covers its 8 batches exactly
    (partial trailing token chunks are fine - tokens ride free/K dims).
    Returns (NVS, order) with order[i*n_cores + c] = original batch
    index processed by core c in slot i."""
    cnt = np.asarray(valid_mask).sum(axis=1)
    order = np.argsort(-cnt, kind="stable")
    NVS = []
    for i in range(BL):
        grp = cnt[order[i * n_cores : (i + 1) * n_cores]]
        NVS.append(max(2, int(grp.max())))
    return tuple(NVS), order


def build_kernel(nc: bass.Bass, cfg: Cfg, reps: int = 1):
    c = cfg
    # x^T and W1 ship as fp8 hi (e4m3) + residual lo (e5m2); the h matmul
    # runs 3 split terms (xh.Wh + xl.Wh + xh.Wl) in DoubleRow mode, each
    # instruction contracting TWO 128-deep k-chunks at 0.5 cycles/row -
    # 0.75x the bf16 PE cost at better-than-bf16 accuracy (the dropped
    # xl.Wl term is ~2^-8 relative).
    xth_d = nc.dram_tensor("xth", [c.BL, c.KD, P, c.NV], F8H, kind="ExternalInput").ap()
    xtl_d = nc.dram_tensor("xtl", [c.BL, c.KD, P, c.NV], F8L, kind="ExternalInput").ap()
    xn_d = nc.dram_tensor("xn", [c.BL, c.NV, c.D], BF16, kind="ExternalInput").ap()
    v_d = nc.dram_tensor("v", [c.BL, P, c.NCV], BF16, kind="ExternalInput").ap()
    w1h_d = nc.dram_tensor("w1h", [c.KD, P, c.HF], F8H, kind="ExternalInput").ap()
    w1l_d = nc.dram_tensor("w1l", [c.KD, P, c.HF], F8L, kind="ExternalInput").ap()
    w2_d = nc.dram_tensor("w2", [c.MC, P, c.H], BF16, kind="ExternalInput").ap()
    b1_d = nc.dram_tensor("b1", [c.HF], FP32, kind="ExternalInput").ap()
    wout_d = nc.dram_tensor("wout", [c.KOUT, P, c.D], BF16, kind="ExternalInput").ap()
    bout_d = nc.dram_tensor("bout", [P, c.KD * c.BL], FP32, kind="ExternalInput").ap()
    y_d = nc.dram_tensor("y", [c.BL, c.D], FP32, kind="ExternalOutput").ap()

    with tile.TileContext(nc) as tc:
        with (
            tc.tile_pool(name="const", bufs=1) as const,
            tc.tile_pool(name="xT", bufs=3) as xT_pool,
            tc.tile_pool(name="h", bufs=4) as h_pool,
            tc.tile_pool(name="xn", bufs=3) as xn_pool,
            tc.tile_pool(name="e", bufs=6) as e_pool,
            tc.tile_pool(name="small", bufs=8) as small_pool,
            tc.tile_pool(name="ysb", bufs=1) as ysb_pool,
            tc.tile_pool(name="hps", bufs=4, space="PSUM") as hps_pool,
            tc.tile_pool(name="scps", bufs=2, space="PSUM") as scps_pool,
            tc.tile_pool(name="plps", bufs=2, space="PSUM") as plps_pool,
            tc.tile_pool(name="acc", bufs=2) as acc_pool,
        ):
            # ---- constants / weights ----
            # W1 as 4 column-quarter tiles: the first matmul group only
            # waits for quarter 0 (~1MB); the rest stream in behind it.
            QW = c.HF // 4
            QMC = c.MC // 4  # f-chunks per quarter
            w1qh = [
                const.tile([P, c.KD, QW], F8H, tag=f"w1qh{q}", name=f"w1qh{q}")
                for q in range(4)
            ]
            w1ql = [
                const.tile([P, c.KD, QW], F8L, tag=f"w1ql{q}", name=f"w1ql{q}")
                for q in range(4)
            ]

            def w1_piece(q, lo, eng=None):
                # one fp8 quarter (0.5MB, 512B innermost -> full DMA rate);
                # hi quarters stream on sync, lo on gpsimd, concurrently
                dst, src = (w1ql[q], w1l_d) if lo else (w1qh[q], w1h_d)
                (eng or nc.sync).dma_start(
                    dst[:],
                    src[:, :, ts(q, QW)].rearrange("k p f -> p k f"),
                )

            w1_piece(0, 0)
            w2_sb = const.tile([P, c.MC, c.H], BF16)
            b1_sb = const.tile([P, c.MC], FP32)
            v_sb = const.tile([P, c.BL, c.NCV], BF16)
            bout_sb = const.tile([P, c.KD * c.BL], FP32)
            # pooled rows at r = b*32 + h (32-aligned per batch for DVE)
            # pooled^T accumulates directly in [128 d, (dc, h)] tiles; the
            # out-projection picks per-(dc,h) batch columns from poolT_sb
            poolT_sb = const.tile([P, c.KD, c.BL, c.H], BF16)
            ones_f32 = const.tile([1, P], FP32)
            nc.gpsimd.memset(ones_f32[:], 1.0)
            wout_sb = const.tile([P, c.KOUT, c.D], BF16)

            pending = []  # deferred (other-block) op emitters

            def flush(limit=0):
                # keep `limit` closures queued: pool matmuls lag their
                # sub-block by a few positions so early ones never stall
                # the PE on the xn DMA
                while len(pending) > limit:
                    pending.pop(0)()

            # flat chunk schedule across batches; xT DMAs prefetch 2 ahead
            sched = []
            for b in range(c.BL):
                t0 = 0
                for bi, TB in enumerate(c.blocks(b)):
                    sched.append((b, bi, t0, TB))
                    t0 += TB
            xts = {}

            def ensure_xt(k):
                if k >= len(sched) or k in xts:
                    return
                b, bi, t0, TB = sched[k]
                xTh = xT_pool.tile([P, c.KD, 512], F8H, tag="xth", name=f"xTh{k % 3}")
                xTl = xT_pool.tile([P, c.KD, 512], F8L, tag="xtl", name=f"xTl{k % 3}")
                if k == 0:
                    # first xt tiles on the scalar queue: transfer
                    # concurrently with w1's quarters on sync/gpsimd
                    nc.scalar.dma_start(
                        xTh[:, :, 0:TB],
                        xth_d[b, :, :, t0 : t0 + TB].rearrange("k p t -> p k t"),
                    )
                    nc.scalar.dma_start(
                        xTl[:, :, 0:TB],
                        xtl_d[b, :, :, t0 : t0 + TB].rearrange("k p t -> p k t"),
                    )
                    # small weights behind them on the scalar queue
                    nc.scalar.dma_start(
                        b1_sb[:], b1_d.rearrange("(c p) -> p c", p=P)
                    )
                    nc.scalar.dma_start(
                        w2_sb[:], w2_d.rearrange("c p h -> p c h")
                    )
                    nc.scalar.dma_start(
                        v_sb[:], v_d.rearrange("b p c -> p b c")
                    )
                    nc.scalar.dma_start(bout_sb[:], bout_d)
                    # rest of W1: hi quarters on sync, lo on gpsimd
                    w1_piece(0, 1, nc.gpsimd)
                    for q in range(1, 4):
                        w1_piece(q, 0)
                        w1_piece(q, 1, nc.gpsimd)
                else:
                    nc.sync.dma_start(
                        xTh[:, :, 0:TB],
                        xth_d[b, :, :, t0 : t0 + TB].rearrange("k p t -> p k t"),
                    )
                    nc.sync.dma_start(
                        xTl[:, :, 0:TB],
                        xtl_d[b, :, :, t0 : t0 + TB].rearrange("k p t -> p k t"),
                    )
                xts[k] = (xTh, xTl)

            kprev = -1
            for b in range(c.BL):
                # pl/z PSUM tiles are bufs=1: allocate only after the
                # previous batch's deferred consumers have been emitted
                # (first flush of this batch), so buffer-reuse tracking
                # sees ops in order.
                z_ps = acc_sb = None
                NCVb = -(-c.NVS[b] // P)
                for bi, TB in enumerate(c.blocks(b)):
                    kprev += 1
                    k = kprev
                    _, _, t0, _ = sched[k]
                    cn0 = t0 // P
                    first = k == 0 and TB == 512
                    ensure_xt(k)
                    ensure_xt(k + 1)
                    ensure_xt(k + 2)
                    xTh, xTl = xts.pop(k)
                    # xn rides the (otherwise idle) gpsimd DMA queue so it
                    # never contends with the xt/wout stream on sync
                    xnt = xn_pool.tile([P, 4, c.D], BF16)
                    fullt = (TB // P) * P  # whole 128-token chunks
                    if fullt:
                        nc.gpsimd.dma_start(
                            xnt[:, 0 : TB // P, :],
                            xn_d[b, t0 : t0 + fullt, :].rearrange(
                                "(s p) d -> p s d", p=P
                            ),
                        )
                    if TB % P:
                        nc.gpsimd.dma_start(
                            xnt[0 : TB % P, TB // P, :],
                            xn_d[b, t0 + fullt : t0 + TB, :],
                        )
                    if b == 1 and bi == 0:
                        # prefetch the output projection during the middle
                        KQ = c.KOUT // 4
                        for q in range(4):
                            nc.sync.dma_start(
                                wout_sb[:, ts(q, KQ), :],
                                wout_d[ts(q, KQ)].rearrange("k p f -> p k f"),
                            )
                    # 128-token sub-blocks processed in PAIRS: the 3 fp8
                    # split terms accumulate per sub as sequential PSUM
                    # groups in one tile, then ONE tanh covers the pair
                    # (halving the Act engine's fixed access cost).
                    subs = -(-TB // P)
                    wid = [min(P, TB - s * P) for s in range(subs)]
                    groups = [
                        list(range(i, min(i + 2, subs)))
                        for i in range(0, subs, 2)
                    ]
                    for group in groups:
                        offs = {}
                        o = 0
                        for s in group:
                            offs[s] = o
                            o += wid[s]
                        wt = o  # total pair width
                        scs, prevq = {}, []
                        for s in group:
                            scs[s] = scps_pool.tile(
                                [P, 512], FP32, tag="sc_ps", name=f"sc{s}"
                            )

                        def dot(s, mc, h_sb):
                            nc.tensor.matmul(
                                scs[s][0 : wid[s], 0 : c.H],
                                h_sb[:, offs[s] : offs[s] + wid[s]],
                                w2_sb[:, mc, :],
                                start=(mc == 0),
                                stop=(mc == c.MC - 1),
                            )

                        for mc in range(c.MC):
                            h_ps = hps_pool.tile([P, 512], FP32, tag="h_ps")
                            wcol = (mc % QMC) * P
                            q = mc // QMC
                            for s in group:
                                w, off = wid[s], offs[s]
                                sp = s * P
                                for t, (lh, rh) in enumerate((
                                    (w1qh[q], xTh), (w1qh[q], xTl),
                                    (w1ql[q], xTh),
                                )):
                                    for pr in range(c.KD // 2):
                                        dc = 2 * pr
                                        nc.tensor.matmul(
                                            h_ps[:, off : off + w],
                                            lh[:, dc : dc + 2, wcol : wcol + P],
                                            rh[:, dc : dc + 2, sp : sp + w],
                                            start=(t == 0 and pr == 0),
                                            stop=(t == 2 and pr == c.KD // 2 - 1),
                                            perf_mode=DR,
                                        )
                            h_sb = h_pool.tile([P, 2 * P], BF16, tag="h_sb")
                            nc.scalar.activation(
                                h_sb[:, 0:wt], h_ps[:, 0:wt], AFT.Tanh,
                                bias=b1_sb[:, mc : mc + 1],
                            )
                            prevq.append((mc, h_sb))
                            if len(prevq) > 2:
                                pm, ph = prevq.pop(0)
                                for s in group:
                                    dot(s, pm, ph)
                            if mc == 2:
                                # drain fully in the kernel's last chunk so
                                # deferred pool work doesn't pile into the tail
                                flush(0 if k == len(sched) - 1 else 3)
                        while prevq:
                            pm, ph = prevq.pop(0)
                            for s in group:
                                dot(s, pm, ph)
                        if acc_sb is None:
                            # full flush: the previous batch's pool/finish
                            # closures must be emitted before the acc
                            # buffer rotates to this batch
                            flush()
                            acc_sb = acc_pool.tile(
                                [P, c.KD * c.H + c.H], FP32, tag="acc"
                            )
                        for s in group:
                            w = wid[s]
                            e_blk = e_pool.tile([P, c.H], BF16, tag="e_blk")
                            nc.scalar.activation(
                                e_blk[0:w, :], scs[s][0:w, 0 : c.H],
                                AFT.Exp, bias=0.0,
                            )

                            def mk_pool(b, s, w, cn, ncv, e_blk, xnt, acc_sb):
                                def go():
                                    # pooled^T chunk: [128 d, H] per dc -
                                    # free dim 4, nearly free on the PE -
                                    # plus Z[h] = v . e on partition 0;
                                    # DVE accumulates chunks across cn
                                    NA = c.KD * c.H
                                    plw = plps_pool.tile(
                                        [P, 512], FP32, tag="plw", name="plw"
                                    )
                                    for dc in range(c.KD):
                                        nc.tensor.matmul(
                                            plw[:, dc * c.H : (dc + 1) * c.H],
                                            xnt[0:w, s, ts(dc, P)],
                                            e_blk[0:w, :],
                                            start=True,
                                            stop=True,
                                        )
                                    nc.tensor.matmul(
                                        plw[0:1, NA : NA + c.H],
                                        v_sb[0:w, b, cn : cn + 1],
                                        e_blk[0:w, :],
                                        start=True,
                                        stop=True,
                                    )
                                    if cn == 0:
                                        nc.vector.tensor_copy(
                                            acc_sb[:, 0:NA], plw[:, 0:NA]
                                        )
                                        nc.vector.tensor_copy(
                                            acc_sb[0:1, NA : NA + c.H],
                                            plw[0:1, NA : NA + c.H],
                                        )
                                    else:
                                        nc.vector.tensor_add(
                                            acc_sb[:, 0:NA], acc_sb[:, 0:NA],
                                            plw[:, 0:NA],
                                        )
                                        nc.vector.tensor_add(
                                            acc_sb[0:1, NA : NA + c.H],
                                            acc_sb[0:1, NA : NA + c.H],
                                            plw[0:1, NA : NA + c.H],
                                        )
                                return go

                            pending.append(
                                mk_pool(b, s, w, cn0 + s, NCVb, e_blk, xnt,
                                        acc_sb)
                            )
                    t0 += TB

                def mk_finish(b, acc_sb):
                    def go():
                        NA = c.KD * c.H
                        rz = small_pool.tile([1, c.H], FP32, tag="rz")
                        nc.vector.reciprocal(rz[:], acc_sb[0:1, NA : NA + c.H])
                        # broadcast rz down the partitions via the PE, then
                        # scale acc into poolT_sb one head-column at a time
                        rzb_ps = scps_pool.tile(
                            [P, 512], FP32, tag="sc_ps", name="rzb"
                        )
                        nc.tensor.matmul(
                            rzb_ps[:, 0 : c.H], ones_f32[:], rz[:],
                            start=True, stop=True,
                        )
                        rzb = small_pool.tile([P, c.H], FP32, tag="rzb")
                        nc.vector.tensor_copy(rzb[:], rzb_ps[:, 0 : c.H])
                        accv = acc_sb[:, 0:NA].rearrange(
                            "p (k h) -> p h k", h=c.H
                        )
                        for hd in range(c.H):
                            nc.vector.tensor_scalar_mul(
                                poolT_sb[:, :, b, hd],
                                accv[:, hd, :],
                                rzb[:, hd : hd + 1],
                            )
                    return go

                pending.append(mk_finish(b, acc_sb))

            flush()
            # ---- output projection y = pooled @ Wout + bout ----
            # y^T chunks [128 d_out, BL]: free dim = BL = 4, so the 256
            # matmuls cost the PE almost nothing (vs 64 x 512-free rows
            # the straight orientation would charge). Groups sequential
            # per d_out chunk -> one PSUM region suffices.
            yt_ps = plps_pool.tile([P, 512], FP32, tag="plw", name="yt_ps")
            ytcols = yt_ps[:, 0 : c.BL * c.KD].rearrange(
                "p (b o) -> p o b", o=c.KD
            )
            for do in range(c.KD):
                for hd in range(c.H):
                    for dc in range(c.KD):
                        nc.tensor.matmul(
                            ytcols[:, do, :],
                            wout_sb[:, hd * c.KD + dc, ts(do, P)],
                            poolT_sb[:, dc, :, hd],
                            start=(hd == 0 and dc == 0),
                            stop=(hd == c.H - 1 and dc == c.KD - 1),
                        )
            # += bout (pre-laid as [P, KD*BL]), then a strided DMA writes
            # y directly in [BL, D] order - no transposes, no extra copy
            yt_sb = ysb_pool.tile([P, c.KD * c.BL], FP32)
            nc.vector.tensor_add(
                yt_sb[:], yt_ps[:, 0 : c.KD * c.BL], bout_sb[:]
            )
            nc.sync.dma_start(
                y_d.rearrange("b (o p) -> p b o", p=P),
                yt_sb[:].rearrange("p (b o) -> p b o", o=c.KD),
            )
    return nc


def make_in_maps(x, valid_mask, W1, b1, W2, b2, Wout, bout, n_cores, cfg,
                 order=None):
    """Host-side prep: compact valid tokens, shard over batch, cast/layout."""
    c = cfg
    bf16 = ml_dtypes.bfloat16
    e4 = ml_dtypes.float8_e4m3
    e5 = ml_dtypes.float8_e5m2
    B = x.shape[0]
    # W1 split into fp8 hi + e5m2 residual (used in DoubleRow mode)
    w1f = np.ascontiguousarray(
        W1.transpose(1, 0, 2).reshape(c.KD, P, c.HF).astype(np.float32)
    )
    w1h_l = np.ascontiguousarray(w1f.astype(e4))
    w1l_l = np.ascontiguousarray(
        (w1f - w1h_l.astype(np.float32)).astype(e5)
    )
    w2f = W2.reshape(c.HF).astype(np.float32)
    w2_l = np.zeros((c.MC, P, c.H), np.float32)
    FC = c.MC // c.H  # f-chunks per head
    for mc in range(c.MC):
        w2_l[mc, :, mc // FC] = w2f[mc * P : (mc + 1) * P]
    w2_l = np.ascontiguousarray(w2_l.astype(bf16))
    b1_l = np.ascontiguousarray(b1.reshape(c.HF).astype(np.float32))
    wout_l = np.ascontiguousarray(Wout.reshape(c.KOUT, P, c.D).astype(bf16))
    # bout pre-laid for the y^T tail: bout_l[p, b*KD + do] = bout[do*128+p]
    bout_l = np.ascontiguousarray(
        np.tile(bout.astype(np.float32).reshape(c.KD, P).T, (1, c.BL))
    )
    # b2 is a per-row constant under the softmax -> it cancels; drop it.
    if order is None:
        order = np.arange(B)
    xc32 = np.zeros((B, c.NV, c.D), np.float32)
    v = np.zeros((B, c.NCV * P), np.float32)
    for gb in range(B):
        idx = np.flatnonzero(valid_mask[gb])[: c.NV]
        xc32[gb, : len(idx)] = x[gb, idx]
        v[gb, : len(idx)] = 1.0
    v_l = np.ascontiguousarray(
        v.reshape(B, c.NCV, P).transpose(0, 2, 1).astype(bf16)
    )
    xc = xc32.astype(bf16)  # pooling operand
    # x^T split into fp8 hi + e5m2 residual for the score matmul
    xt32 = np.ascontiguousarray(xc32.transpose(0, 2, 1)).reshape(
        B, c.KD, P, c.NV
    )
    xth_all = np.ascontiguousarray(xt32.astype(e4))
    xtl_all = np.ascontiguousarray(
        (xt32 - xth_all.astype(np.float32)).astype(e5)
    )
    in_maps = []
    for core in range(n_cores):
        sel = [order[i * n_cores + core] for i in range(c.BL)]
        in_maps.append(
            {
                "xth": np.ascontiguousarray(xth_all[sel]),
                "xtl": np.ascontiguousarray(xtl_all[sel]),
                "xn": np.ascontiguousarray(xc[sel]),
                "v": np.ascontiguousarray(v_l[sel]),
                "w1h": w1h_l,
                "w1l": w1l_l,
                "w2": w2_l,
                "b1": b1_l,
                "wout": wout_l,
                "bout": bout_l,
            }
        )
    return in_maps


_cached = {}
last_results = None


def kernel(x, valid_mask, W1, b1, W2, b2, Wout, bout, trace=False):
    global last_results
    x, valid_mask, W1, b1, W2, b2, Wout, bout = (
        np.asarray(a)
        for a in (x, valid_mask, W1, b1, W2, b2, Wout, bout)
    )
    B = x.shape[0]
    n_cores = 8
    NVS, order = choose_slots(valid_mask, n_cores, B // n_cores)
    cfg = Cfg(BL=B // n_cores, NVS=NVS)
    key = (B, NVS)
    if key not in _cached:
        nc = bacc.Bacc("TRN2", target_bir_lowering=False, debug=False)
        build_kernel(nc, cfg)
        nc.compile()
        _cached[key] = nc
    in_maps = make_in_maps(
        x, valid_mask, W1, b1, W2, b2, Wout, bout, n_cores, cfg, order
    )
    res = run_bass_kernel_spmd(
        _cached[key], in_maps, core_ids=list(range(n_cores)), trace=trace
    )
    last_results = res
    y = np.empty((B, x.shape[2]), np.float32)
    for core in range(n_cores):
        yc = np.asarray(res.results[core]["y"], np.float32)
        for i in range(cfg.BL):
            y[order[i * n_cores + core]] = yc[i]
    return y


# revision 89
# speedup vs baseline: 1.8104x; 1.4028x over previous
"""AttentionPool Trainium2 Bass kernel (valid-token compaction).

Reference computation (per batch b):
    h      = tanh(x @ W1 + b1)          # [N, H*F]   (big matmul, bf16 on PE)
    scores = h @ W2 + b2                # [N, H]
    scores = where(mask, scores, -1e9)
    w      = softmax(scores, axis=N)    # per head
    pooled = w.T @ x                    # [H, D]
    y      = concat_h(pooled) @ Wout + bout   # [D]

Key structural ideas:
  - Invalid tokens get softmax weight 0 and contribute nothing to the
    output, so the host compacts each batch's valid tokens (~1024 of 2048
    at p=0.5) into a contiguous buffer and the big x@W1 matmul runs only
    on those (~2x less PE work). Batches are sorted by valid count and
    assigned to per-slot lengths NVS (exact, partial trailing 128-token
    chunks are fine), so every core does the same near-minimal work.
  - Padding inside a slot carries x=0: it cannot pollute the pooling sum,
    and the softmax denominator is computed as a validity-weighted PE
    reduction Z = sum_t v_t e_t (v in {1,0}), so no -1e9 mask tensor or
    max-shift exists anywhere. b2 cancels under softmax and is dropped.
  - All small-output reductions are shaped so the PE streams tiny free
    dims (matmul cost ~ out free size): the score dot emits [tok, H=4]
    tiles, pooling emits pooled^T [128d, H=4] tiles per (chunk, dc) that
    the DVE accumulates across chunks, and the output projection emits
    y^T [128d, B=4] columns. Everything lands pre-transposed for its
    consumer; the only PE transposes left are none.
  - The exp(scores) tiles are already token-major = the pooling lhsT.

Sharding: data-parallel over batch B=32 across 8 cores (4 batches/core,
count-sorted slot assignment, results un-permuted on the host). Weights
replicated. Matmuls bf16 (fp32 PSUM); softmax fp32 on the Act engine
(the exp_and_others act table holds both tanh and exp - no table loads).
|scores| <= ||W2||_1 ~ 18, so exp is safe without max-shift.

Pipelining (keeps the PE dense, which also keeps its p-state ramped):
  - per f-chunk mc: the 8 k-matmuls of chunk mc+1..mc+2 are issued before
    the score dot of chunk mc (depth-2 queue), hiding the tanh latency.
  - a sub-block's pooling/Z matmuls are deferred 3 sub-blocks down the
    instruction stream, when exp and the xn DMA have long finished.
  - W1 streams in 256-column half-quarters on two DMA queues (sync +
    gpsimd) paced to the first chunk's interleaved sub-pair consumption;
    xT prefetches 2 chunks ahead; xn rides the gpsimd queue.
  - narrow trailing sub-blocks are interleave-paired with a full one so
    PSUM h-buffer recycling never waits on the tanh pipeline.
"""

import numpy as np
import ml_dtypes

import concourse.bass as bass
import concourse.mybir as mybir
import concourse.tile as tile
from concourse import bacc
from concourse.bass import ts
from concourse.bass_utils import run_bass_kernel_spmd
BF16 = mybir.dt.bfloat16
FP32 = mybir.dt.float32
F8H = mybir.dt.float8e4
F8L = mybir.dt.float8e5
DR = mybir.MatmulPerfMode.DoubleRow
AFT = mybir.ActivationFunctionType

P = 128
# fp8 split of the score matmul: 2 terms = xh.Wh + xl.Wh (W1 pre-scaled
# by W1SCALE, exactly undone by the tanh's scale; measured end-to-end rel
# err 1.35e-2 vs the 2e-2 gate). Set to 3 to add the xh.Wl term
# (rel 1.8e-3) at 1.5x the PE cost of 2 terms.
W1TERMS = 2
W1SCALE = 64.0


class Cfg:
    def __init__(self, BL=4, NVS=(1152,) * 4, D=1024, H=4, F=512):
        # NVS: per-batch-slot padded valid-token count (batches are
        # assigned to slots sorted by count, so later slots can be shorter)
        self.BL, self.NVS, self.D, self.H, self.F = BL, tuple(NVS), D, H, F
        assert len(self.NVS) == BL
        self.NV = max(self.NVS)   # shipped tensor size
        self.HF = H * F
        self.KD = D // P          # k-chunks of D
        self.MC = self.HF // P    # f-chunks of H*F
        self.NCV = -(-self.NV // P)  # token chunks (max slot, ceil)
        self.KOUT = (H * D) // P  # k-chunks of the output projection

    def blocks(self, b):
        """DMA token blocks for batch slot b: 512-wide + remainder. A
        short (<=128) remainder is merged with 128 tokens borrowed from
        the previous block so the final chunk has two sub-blocks - the
        narrow one is interleave-paired with a full one to keep the PSUM
        buffer-recycle window wider than the tanh pipeline latency."""
        nv = self.NVS[b]
        out = [512] * (nv // 512)
        r = nv % 512
        if r:
            if r <= P and out:
                out[-1] -= P
                r += P
            out.append(r)
        return out


def choose_slots(valid_mask: np.ndarray, n_cores=8, BL=4):
    """Sort batches by valid count; slot i takes ranks [i*n_cores,
    (i+1)*n_cores) so each slot's NV covers its 8 batches exactly
    (partial trailing token chunks are fine - tokens ride free/K dims).
    Returns (NVS, order) with order[i*n_cores + c] = original batch
    index processed by core c in slot i."""
    cnt = np.asarray(valid_mask).sum(axis=1)
    order = np.argsort(-cnt, kind="stable")
    NVS = []
    for i in range(BL):
        grp = cnt[order[i * n_cores : (i + 1) * n_cores]]
        NVS.append(max(2, int(grp.max())))
    return tuple(NVS), order


def build_kernel(nc: bass.Bass, cfg: Cfg, reps: int = 1):
    c = cfg
    # x^T and W1 ship as fp8 hi (e4m3) + residual lo (e5m2); the h matmul
    # runs 3 split terms (xh.Wh + xl.Wh + xh.Wl) in DoubleRow mode, each
    # instruction contracting TWO 128-deep k-chunks at 0.5 cycles/row -
    # 0.75x the bf16 PE cost at better-than-bf16 accuracy (the dropped
    # xl.Wl term is ~2^-8 relative).
    xth_d = nc.dram_tensor("xth", [c.BL, c.KD, P, c.NV], F8H, kind="ExternalInput").ap()
    xtl_d = nc.dram_tensor("xtl", [c.BL, c.KD, P, c.NV], F8L, kind="ExternalInput").ap()
    xn_d = nc.dram_tensor("xn", [c.BL, c.NV, c.D], BF16, kind="ExternalInput").ap()
    v_d = nc.dram_tensor("v", [c.BL, P, c.NCV], BF16, kind="ExternalInput").ap()
    w1h_d = nc.dram_tensor("w1h", [c.KD, P, c.HF], F8H, kind="ExternalInput").ap()
    w1l_d = nc.dram_tensor("w1l", [c.KD, P, c.HF], F8L, kind="ExternalInput").ap()
    w2_d = nc.dram_tensor("w2", [c.MC, P, c.H], BF16, kind="ExternalInput").ap()
    b1_d = nc.dram_tensor("b1", [c.HF], FP32, kind="ExternalInput").ap()
    wout_d = nc.dram_tensor("wout", [c.KOUT, P, c.D], BF16, kind="ExternalInput").ap()
    bout_d = nc.dram_tensor("bout", [P, c.KD * c.BL], FP32, kind="ExternalInput").ap()
    y_d = nc.dram_tensor("y", [c.BL, c.D], FP32, kind="ExternalOutput").ap()

    with tile.TileContext(nc) as tc:
        with (
            tc.tile_pool(name="const", bufs=1) as const,
            tc.tile_pool(name="xT", bufs=3) as xT_pool,
            tc.tile_pool(name="h", bufs=4) as h_pool,
            tc.tile_pool(name="xn", bufs=3) as xn_pool,
            tc.tile_pool(name="e", bufs=10) as e_pool,
            tc.tile_pool(name="small", bufs=8) as small_pool,
            tc.tile_pool(name="ysb", bufs=1) as ysb_pool,
            tc.tile_pool(name="hps", bufs=3, space="PSUM") as hps_pool,
            tc.tile_pool(name="scps", bufs=4, space="PSUM") as scps_pool,
            tc.tile_pool(name="plps", bufs=1, space="PSUM") as plps_pool,
            tc.tile_pool(name="acc", bufs=2) as acc_pool,
        ):
            # ---- constants / weights ----
            # W1 as 4 column-quarter tiles: the first matmul group only
            # waits for quarter 0 (~1MB); the rest stream in behind it.
            QW = c.HF // 4
            QMC = c.MC // 4  # f-chunks per quarter
            w1qh = [
                const.tile([P, c.KD, QW], F8H, tag=f"w1qh{q}", name=f"w1qh{q}")
                for q in range(4)
            ]
            w1ql = [
                const.tile([P, c.KD, QW], F8L, tag=f"w1ql{q}", name=f"w1ql{q}")
                for q in range(4)
            ]

            def w1_piece(q, lo, eng=None):
                # one fp8 quarter (0.5MB, 512B innermost -> full DMA rate);
                # hi quarters stream on sync, lo on gpsimd, concurrently
                dst, src = (w1ql[q], w1l_d) if lo else (w1qh[q], w1h_d)
                (eng or nc.sync).dma_start(
                    dst[:],
                    src[:, :, ts(q, QW)].rearrange("k p f -> p k f"),
                )

            w1_piece(0, 0)
            w2_sb = const.tile([P, c.MC, c.H], BF16)
            b1_sb = const.tile([P, c.MC], FP32)
            v_sb = const.tile([P, c.BL, c.NCV], BF16)
            bout_sb = const.tile([P, c.KD * c.BL], FP32)
            # pooled rows at r = b*32 + h (32-aligned per batch for DVE)
            # pooled^T accumulates directly in [128 d, (dc, h)] tiles; the
            # out-projection picks per-(dc,h) batch columns from poolT_sb
            poolT_sb = const.tile([P, c.KD, c.BL, c.H], BF16)
            ones_f32 = const.tile([1, P], FP32)
            nc.gpsimd.memset(ones_f32[:], 1.0)
            wout_sb = const.tile([P, c.KOUT, c.D], BF16)

            pending = []  # deferred (other-block) op emitters

            def flush(limit=0):
                # keep `limit` closures queued: pool matmuls lag their
                # sub-block by a few positions so early ones never stall
                # the PE on the xn DMA
                while len(pending) > limit:
                    pending.pop(0)()

            # flat chunk schedule across batches; xT DMAs prefetch 2 ahead
            sched = []
            for b in range(c.BL):
                t0 = 0
                for bi, TB in enumerate(c.blocks(b)):
                    sched.append((b, bi, t0, TB))
                    t0 += TB
            xts = {}

            def ensure_xt(k):
                if k >= len(sched) or k in xts:
                    return
                b, bi, t0, TB = sched[k]
                xTh = xT_pool.tile([P, c.KD, 512], F8H, tag="xth", name=f"xTh{k % 3}")
                xTl = xT_pool.tile([P, c.KD, 512], F8L, tag="xtl", name=f"xTl{k % 3}")
                if k == 0:
                    # first xt tiles on the scalar queue: transfer
                    # concurrently with w1's quarters on sync/gpsimd
                    nc.scalar.dma_start(
                        xTh[:, :, 0:TB],
                        xth_d[b, :, :, t0 : t0 + TB].rearrange("k p t -> p k t"),
                    )
                    nc.scalar.dma_start(
                        xTl[:, :, 0:TB],
                        xtl_d[b, :, :, t0 : t0 + TB].rearrange("k p t -> p k t"),
                    )
                    # small weights behind them on the scalar queue
                    nc.scalar.dma_start(
                        b1_sb[:], b1_d.rearrange("(c p) -> p c", p=P)
                    )
                    nc.scalar.dma_start(
                        w2_sb[:], w2_d.rearrange("c p h -> p c h")
                    )
                    nc.scalar.dma_start(
                        v_sb[:], v_d.rearrange("b p c -> p b c")
                    )
                    nc.scalar.dma_start(bout_sb[:], bout_d)
                    # rest of W1: hi quarters on sync, lo on gpsimd
                    if W1TERMS >= 3:
                        w1_piece(0, 1, nc.gpsimd)
                    for q in range(1, 4):
                        w1_piece(q, 0)
                        if W1TERMS >= 3:
                            w1_piece(q, 1, nc.gpsimd)
                else:
                    nc.sync.dma_start(
                        xTh[:, :, 0:TB],
                        xth_d[b, :, :, t0 : t0 + TB].rearrange("k p t -> p k t"),
                    )
                    nc.sync.dma_start(
                        xTl[:, :, 0:TB],
                        xtl_d[b, :, :, t0 : t0 + TB].rearrange("k p t -> p k t"),
                    )
                xts[k] = (xTh, xTl)

            kprev = -1
            for b in range(c.BL):
                # pl/z PSUM tiles are bufs=1: allocate only after the
                # previous batch's deferred consumers have been emitted
                # (first flush of this batch), so buffer-reuse tracking
                # sees ops in order.
                z_ps = acc_sb = None
                NCVb = -(-c.NVS[b] // P)
                for bi, TB in enumerate(c.blocks(b)):
                    kprev += 1
                    k = kprev
                    _, _, t0, _ = sched[k]
                    cn0 = t0 // P
                    first = k == 0 and TB == 512
                    ensure_xt(k)
                    ensure_xt(k + 1)
                    ensure_xt(k + 2)
                    xTh, xTl = xts.pop(k)
                    # xn rides the (otherwise idle) gpsimd DMA queue so it
                    # never contends with the xt/wout stream on sync
                    xnt = xn_pool.tile([P, 4, c.D], BF16)
                    fullt = (TB // P) * P  # whole 128-token chunks
                    if fullt:
                        nc.gpsimd.dma_start(
                            xnt[:, 0 : TB // P, :],
                            xn_d[b, t0 : t0 + fullt, :].rearrange(
                                "(s p) d -> p s d", p=P
                            ),
                        )
                    if TB % P:
                        nc.gpsimd.dma_start(
                            xnt[0 : TB % P, TB // P, :],
                            xn_d[b, t0 + fullt : t0 + TB, :],
                        )
                    if b == 1 and bi == 0:
                        # prefetch the output projection during the middle
                        KQ = c.KOUT // 4
                        for q in range(4):
                            nc.sync.dma_start(
                                wout_sb[:, ts(q, KQ), :],
                                wout_d[ts(q, KQ)].rearrange("k p f -> p k f"),
                            )
                    # 128-token sub-blocks processed in PAIRS: the 3 fp8
                    # split terms accumulate per sub as sequential PSUM
                    # groups in one tile, then ONE tanh covers the pair
                    # (halving the Act engine's fixed access cost).
                    subs = -(-TB // P)
                    wid = [min(P, TB - s * P) for s in range(subs)]
                    groups = [
                        list(range(i, min(i + 4, subs)))
                        for i in range(0, subs, 4)
                    ]
                    for group in groups:
                        offs = {}
                        o = 0
                        for s in group:
                            offs[s] = o
                            o += wid[s]
                        wt = o  # total pair width
                        scs, prevq = {}, []
                        for s in group:
                            scs[s] = scps_pool.tile(
                                [P, 512], FP32, tag="sc_ps", name=f"sc{s}"
                            )

                        def dot(s, mc, h_sb):
                            nc.tensor.matmul(
                                scs[s][0 : wid[s], 0 : c.H],
                                h_sb[:, offs[s] : offs[s] + wid[s]],
                                w2_sb[:, mc, :],
                                start=(mc == 0),
                                stop=(mc == c.MC - 1),
                            )

                        for mc in range(c.MC):
                            h_ps = hps_pool.tile([P, 512], FP32, tag="h_ps")
                            wcol = (mc % QMC) * P
                            q = mc // QMC
                            terms = [(w1qh[q], xTh), (w1qh[q], xTl),
                                     (w1ql[q], xTh)][:W1TERMS]
                            for s in group:
                                w, off = wid[s], offs[s]
                                sp = s * P
                                for t, (lh, rh) in enumerate(terms):
                                    for pr in range(c.KD // 2):
                                        dc = 2 * pr
                                        nc.tensor.matmul(
                                            h_ps[:, off : off + w],
                                            lh[:, dc : dc + 2, wcol : wcol + P],
                                            rh[:, dc : dc + 2, sp : sp + w],
                                            start=(t == 0 and pr == 0),
                                            stop=(
                                                t == len(terms) - 1
                                                and pr == c.KD // 2 - 1
                                            ),
                                            perf_mode=DR,
                                        )
                            h_sb = h_pool.tile([P, 4 * P], BF16, tag="h_sb")
                            nc.scalar.activation(
                                h_sb[:, 0:wt], h_ps[:, 0:wt], AFT.Tanh,
                                bias=b1_sb[:, mc : mc + 1],
                                scale=1.0 / W1SCALE,
                            )
                            prevq.append((mc, h_sb))
                            if len(prevq) > 2:
                                pm, ph = prevq.pop(0)
                                for s in group:
                                    dot(s, pm, ph)
                            if mc == 2:
                                # drain fully in the kernel's last chunk so
                                # deferred pool work doesn't pile into the tail
                                flush(0 if k == len(sched) - 1 else 3)
                        while prevq:
                            pm, ph = prevq.pop(0)
                            for s in group:
                                dot(s, pm, ph)
                        if acc_sb is None:
                            # full flush: the previous batch's pool/finish
                            # closures must be emitted before the acc
                            # buffer rotates to this batch
                            flush()
                            acc_sb = acc_pool.tile(
                                [P, c.KD * c.H + c.H], FP32, tag="acc"
                            )
                        for s in group:
                            w = wid[s]
                            e_blk = e_pool.tile([P, c.H], BF16, tag="e_blk")
                            nc.scalar.activation(
                                e_blk[0:w, :], scs[s][0:w, 0 : c.H],
                                AFT.Exp, bias=0.0,
                            )

                            def mk_pool(b, s, w, cn, ncv, e_blk, xnt, acc_sb):
                                def go():
                                    # pooled^T chunk: [128 d, H] per dc -
                                    # free dim 4, nearly free on the PE -
                                    # plus Z[h] = v . e on partition 0;
                                    # DVE accumulates chunks across cn
                                    NA = c.KD * c.H
                                    plw = plps_pool.tile(
                                        [P, 512], FP32, tag="plw", name="plw"
                                    )
                                    for dc in range(c.KD):
                                        nc.tensor.matmul(
                                            plw[:, dc * c.H : (dc + 1) * c.H],
                                            xnt[0:w, s, ts(dc, P)],
                                            e_blk[0:w, :],
                                            start=True,
                                            stop=True,
                                        )
                                    nc.tensor.matmul(
                                        plw[0:1, NA : NA + c.H],
                                        v_sb[0:w, b, cn : cn + 1],
                                        e_blk[0:w, :],
                                        start=True,
                                        stop=True,
                                    )
                                    if cn == 0:
                                        nc.vector.tensor_copy(
                                            acc_sb[:, 0:NA], plw[:, 0:NA]
                                        )
                                        nc.vector.tensor_copy(
                                            acc_sb[0:1, NA : NA + c.H],
                                            plw[0:1, NA : NA + c.H],
                                        )
                                    else:
                                        nc.vector.tensor_add(
                                            acc_sb[:, 0:NA], acc_sb[:, 0:NA],
                                            plw[:, 0:NA],
                                        )
                                        nc.vector.tensor_add(
                                            acc_sb[0:1, NA : NA + c.H],
                                            acc_sb[0:1, NA : NA + c.H],
                                            plw[0:1, NA : NA + c.H],
                                        )
                                return go

                            pending.append(
                                mk_pool(b, s, w, cn0 + s, NCVb, e_blk, xnt,
                                        acc_sb)
                            )
                    t0 += TB

                def mk_finish(b, acc_sb):
                    def go():
                        NA = c.KD * c.H
                        rz = small_pool.tile([1, c.H], FP32, tag="rz")
                        nc.vector.reciprocal(rz[:], acc_sb[0:1, NA : NA + c.H])
                        # broadcast rz down the partitions via the PE, then
                        # scale acc into poolT_sb one head-column at a time
                        rzb_ps = scps_pool.tile(
                            [P, 512], FP32, tag="sc_ps", name="rzb"
                        )
                        nc.tensor.matmul(
                            rzb_ps[:, 0 : c.H], ones_f32[:], rz[:],
                            start=True, stop=True,
                        )
                        rzb = small_pool.tile([P, c.H], FP32, tag="rzb")
                        nc.vector.tensor_copy(rzb[:], rzb_ps[:, 0 : c.H])
                        accv = acc_sb[:, 0:NA].rearrange(
                            "p (k h) -> p h k", h=c.H
                        )
                        for hd in range(c.H):
                            nc.vector.tensor_scalar_mul(
                                poolT_sb[:, :, b, hd],
                                accv[:, hd, :],
                                rzb[:, hd : hd + 1],
                            )
                    return go

                pending.append(mk_finish(b, acc_sb))

            flush()
            # ---- output projection y = pooled @ Wout + bout ----
            # y^T chunks [128 d_out, BL]: free dim = BL = 4, so the 256
            # matmuls cost the PE almost nothing (vs 64 x 512-free rows
            # the straight orientation would charge). Groups sequential
            # per d_out chunk -> one PSUM region suffices.
            yt_ps = plps_pool.tile([P, 512], FP32, tag="plw", name="yt_ps")
            ytcols = yt_ps[:, 0 : c.BL * c.KD].rearrange(
                "p (b o) -> p o b", o=c.KD
            )
            for do in range(c.KD):
                for hd in range(c.H):
                    for dc in range(c.KD):
                        nc.tensor.matmul(
                            ytcols[:, do, :],
                            wout_sb[:, hd * c.KD + dc, ts(do, P)],
                            poolT_sb[:, dc, :, hd],
                            start=(hd == 0 and dc == 0),
                            stop=(hd == c.H - 1 and dc == c.KD - 1),
                        )
            # += bout (pre-laid as [P, KD*BL]), then a strided DMA writes
            # y directly in [BL, D] order - no transposes, no extra copy
            yt_sb = ysb_pool.tile([P, c.KD * c.BL], FP32)
            nc.vector.tensor_add(
                yt_sb[:], yt_ps[:, 0 : c.KD * c.BL], bout_sb[:]
            )
            nc.sync.dma_start(
                y_d.rearrange("b (o p) -> p b o", p=P),
                yt_sb[:].rearrange("p (b o) -> p b o", o=c.KD),
            )
    return nc


def make_in_maps(x, valid_mask, W1, b1, W2, b2, Wout, bout, n_cores, cfg,
                 order=None):
    """Host-side prep: compact valid tokens, shard over batch, cast/layout."""
    c = cfg
    bf16 = ml_dtypes.bfloat16
    e4 = ml_dtypes.float8_e4m3
    e5 = ml_dtypes.float8_e5m2
    B = x.shape[0]
    # W1 split into fp8 hi + e5m2 residual (used in DoubleRow mode),
    # pre-scaled so its sigma sits well inside e4m3's normal range; the
    # tanh's scale undoes it exactly
    w1f = np.ascontiguousarray(
        W1.transpose(1, 0, 2).reshape(c.KD, P, c.HF).astype(np.float32)
        * np.float32(W1SCALE)
    )
    w1h_l = np.ascontiguousarray(w1f.astype(e4))
    w1l_l = np.ascontiguousarray(
        (w1f - w1h_l.astype(np.float32)).astype(e5)
    )
    w2f = W2.reshape(c.HF).astype(np.float32)
    w2_l = np.zeros((c.MC, P, c.H), np.float32)
    FC = c.MC // c.H  # f-chunks per head
    for mc in range(c.MC):
        w2_l[mc, :, mc // FC] = w2f[mc * P : (mc + 1) * P]
    w2_l = np.ascontiguousarray(w2_l.astype(bf16))
    b1_l = np.ascontiguousarray(b1.reshape(c.HF).astype(np.float32))
    wout_l = np.ascontiguousarray(Wout.reshape(c.KOUT, P, c.D).astype(bf16))
    # bout pre-laid for the y^T tail: bout_l[p, b*KD + do] = bout[do*128+p]
    bout_l = np.ascontiguousarray(
        np.tile(bout.astype(np.float32).reshape(c.KD, P).T, (1, c.BL))
    )
    # b2 is a per-row constant under the softmax -> it cancels; drop it.
    if order is None:
        order = np.arange(B)
    xc32 = np.zeros((B, c.NV, c.D), np.float32)
    v = np.zeros((B, c.NCV * P), np.float32)
    for gb in range(B):
        idx = np.flatnonzero(valid_mask[gb])[: c.NV]
        xc32[gb, : len(idx)] = x[gb, idx]
        v[gb, : len(idx)] = 1.0
    v_l = np.ascontiguousarray(
        v.reshape(B, c.NCV, P).transpose(0, 2, 1).astype(bf16)
    )
    xc = xc32.astype(bf16)  # pooling operand
    # x^T split into fp8 hi + e5m2 residual for the score matmul
    xt32 = np.ascontiguousarray(xc32.transpose(0, 2, 1)).reshape(
        B, c.KD, P, c.NV
    )
    xth_all = np.ascontiguousarray(xt32.astype(e4))
    xtl_all = np.ascontiguousarray(
        (xt32 - xth_all.astype(np.float32)).astype(e5)
    )
    in_maps = []
    for core in range(n_cores):
        sel = [order[i * n_cores + core] for i in range(c.BL)]
        in_maps.append(
            {
                "xth": np.ascontiguousarray(xth_all[sel]),
                "xtl": np.ascontiguousarray(xtl_all[sel]),
                "xn": np.ascontiguousarray(xc[sel]),
                "v": np.ascontiguousarray(v_l[sel]),
                "w1h": w1h_l,
                "w1l": w1l_l,
                "w2": w2_l,
                "b1": b1_l,
                "wout": wout_l,
                "bout": bout_l,
            }
        )
    return in_maps


_cached = {}
last_results = None


def kernel(x, valid_mask, W1, b1, W2, b2, Wout, bout, trace=False):
    global last_results
    x, valid_mask, W1, b1, W2, b2, Wout, bout = (
        np.asarray(a)
        for a in (x, valid_mask, W1, b1, W2, b2, Wout, bout)
    )
    B = x.shape[0]
    n_cores = 8
    NVS, order = choose_slots(valid_mask, n_cores, B // n_cores)
    cfg = Cfg(BL=B // n_cores, NVS=NVS)
    key = (B, NVS)
    if key not in _cached:
        nc = bacc.Bacc("TRN2", target_bir_lowering=False, debug=False)
        build_kernel(nc, cfg)
        nc.compile()
        _cached[key] = nc
    in_maps = make_in_maps(
        x, valid_mask, W1, b1, W2, b2, Wout, bout, n_cores, cfg, order
    )
    res = run_bass_kernel_spmd(
        _cached[key], in_maps, core_ids=list(range(n_cores)), trace=trace
    )
    last_results = res
    y = np.empty((B, x.shape[2]), np.float32)
    for core in range(n_cores):
        yc = np.asarray(res.results[core]["y"], np.float32)
        for i in range(cfg.BL):
            y[order[i * n_cores + core]] = yc[i]
    return y


# revision 99
# speedup vs baseline: 1.8112x; 1.0004x over previous
"""AttentionPool Trainium2 Bass kernel (valid-token compaction).

Reference computation (per batch b):
    h      = tanh(x @ W1 + b1)          # [N, H*F]   (big matmul, bf16 on PE)
    scores = h @ W2 + b2                # [N, H]
    scores = where(mask, scores, -1e9)
    w      = softmax(scores, axis=N)    # per head
    pooled = w.T @ x                    # [H, D]
    y      = concat_h(pooled) @ Wout + bout   # [D]

Key structural ideas:
  - Invalid tokens get softmax weight 0 and contribute nothing to the
    output, so the host compacts each batch's valid tokens (~1024 of 2048
    at p=0.5) into a contiguous buffer and the big x@W1 matmul runs only
    on those (~2x less PE work). Batches are sorted by valid count and
    assigned to per-slot lengths NVS (exact, partial trailing 128-token
    chunks are fine), so every core does the same near-minimal work.
  - Padding inside a slot carries x=0: it cannot pollute the pooling sum,
    and the softmax denominator is computed as a validity-weighted PE
    reduction Z = sum_t v_t e_t (v in {1,0}), so no -1e9 mask tensor or
    max-shift exists anywhere. b2 cancels under softmax and is dropped.
  - All small-output reductions are shaped so the PE streams tiny free
    dims (matmul cost ~ out free size): the score dot emits [tok, H=4]
    tiles, pooling emits pooled^T [128d, H=4] tiles per (chunk, dc) that
    the DVE accumulates across chunks, and the output projection emits
    y^T [128d, B=4] columns. Everything lands pre-transposed for its
    consumer; the only PE transposes left are none.
  - The exp(scores) tiles are already token-major = the pooling lhsT.

Sharding: data-parallel over batch B=32 across 8 cores (4 batches/core,
count-sorted slot assignment, results un-permuted on the host). Weights
replicated. Matmuls bf16 (fp32 PSUM); softmax fp32 on the Act engine
(the exp_and_others act table holds both tanh and exp - no table loads).
|scores| <= ||W2||_1 ~ 18, so exp is safe without max-shift.

Pipelining (keeps the PE dense, which also keeps its p-state ramped):
  - per f-chunk mc: the 8 k-matmuls of chunk mc+1..mc+2 are issued before
    the score dot of chunk mc (depth-2 queue), hiding the tanh latency.
  - a sub-block's pooling/Z matmuls are deferred 3 sub-blocks down the
    instruction stream, when exp and the xn DMA have long finished.
  - W1 streams in 256-column half-quarters on two DMA queues (sync +
    gpsimd) paced to the first chunk's interleaved sub-pair consumption;
    xT prefetches 2 chunks ahead; xn rides the gpsimd queue.
  - narrow trailing sub-blocks are interleave-paired with a full one so
    PSUM h-buffer recycling never waits on the tanh pipeline.
"""

import numpy as np
import ml_dtypes

import concourse.bass as bass
import concourse.mybir as mybir
import concourse.tile as tile
from concourse import bacc
from concourse.bass import ts
from concourse.bass_utils import run_bass_kernel_spmd
BF16 = mybir.dt.bfloat16
FP32 = mybir.dt.float32
F8H = mybir.dt.float8e4
F8L = mybir.dt.float8e5
DR = mybir.MatmulPerfMode.DoubleRow
AFT = mybir.ActivationFunctionType

P = 128
# fp8 split of the score matmul: 2 terms = xh.Wh + xl.Wh (W1 pre-scaled
# by W1SCALE, exactly undone by the tanh's scale; measured end-to-end rel
# err 1.35e-2 vs the 2e-2 gate). Set to 3 to add the xh.Wl term
# (rel 1.8e-3) at 1.5x the PE cost of 2 terms.
W1TERMS = 2
W1SCALE = 64.0


class Cfg:
    def __init__(self, BL=4, NVS=(1152,) * 4, D=1024, H=4, F=512):
        # NVS: per-batch-slot padded valid-token count (batches are
        # assigned to slots sorted by count, so later slots can be shorter)
        self.BL, self.NVS, self.D, self.H, self.F = BL, tuple(NVS), D, H, F
        assert len(self.NVS) == BL
        self.NV = max(self.NVS)   # shipped tensor size
        self.HF = H * F
        self.KD = D // P          # k-chunks of D
        self.MC = self.HF // P    # f-chunks of H*F
        self.NCV = -(-self.NV // P)  # token chunks (max slot, ceil)
        self.KOUT = (H * D) // P  # k-chunks of the output projection

    def blocks(self, b):
        """DMA token blocks for batch slot b: 512-wide + remainder. A
        short (<=128) remainder is merged with 128 tokens borrowed from
        the previous block so the final chunk has two sub-blocks - the
        narrow one is interleave-paired with a full one to keep the PSUM
        buffer-recycle window wider than the tanh pipeline latency."""
        nv = self.NVS[b]
        out = [512] * (nv // 512)
        r = nv % 512
        if r:
            if r <= P and out:
                out[-1] -= P
                r += P
            out.append(r)
        return out


def choose_slots(valid_mask: np.ndarray, n_cores=8, BL=4):
    """Sort batches by valid count; slot i takes ranks [i*n_cores,
    (i+1)*n_cores) so each slot's NV covers its 8 batches exactly
    (partial trailing token chunks are fine - tokens ride free/K dims).
    Returns (NVS, order) with order[i*n_cores + c] = original batch
    index processed by core c in slot i."""
    cnt = np.asarray(valid_mask).sum(axis=1)
    order = np.argsort(-cnt, kind="stable")
    NVS = []
    for i in range(BL):
        grp = cnt[order[i * n_cores : (i + 1) * n_cores]]
        NVS.append(max(2, int(grp.max())))
    return tuple(NVS), order


def build_kernel(nc: bass.Bass, cfg: Cfg, reps: int = 1):
    c = cfg
    # x^T and W1 ship as fp8 hi (e4m3) + residual lo (e5m2); the h matmul
    # runs 3 split terms (xh.Wh + xl.Wh + xh.Wl) in DoubleRow mode, each
    # instruction contracting TWO 128-deep k-chunks at 0.5 cycles/row -
    # 0.75x the bf16 PE cost at better-than-bf16 accuracy (the dropped
    # xl.Wl term is ~2^-8 relative).
    xth_d = nc.dram_tensor("xth", [c.BL, c.KD, P, c.NV], F8H, kind="ExternalInput").ap()
    xtl_d = nc.dram_tensor("xtl", [c.BL, c.KD, P, c.NV], F8L, kind="ExternalInput").ap()
    xn_d = nc.dram_tensor("xn", [c.BL, c.NV, c.D], BF16, kind="ExternalInput").ap()
    v_d = nc.dram_tensor("v", [c.BL, P, c.NCV], BF16, kind="ExternalInput").ap()
    w1h_d = nc.dram_tensor("w1h", [c.KD, P, c.HF], F8H, kind="ExternalInput").ap()
    w1l_d = nc.dram_tensor("w1l", [c.KD, P, c.HF], F8L, kind="ExternalInput").ap()
    w2_d = nc.dram_tensor("w2", [c.MC, P, c.H], BF16, kind="ExternalInput").ap()
    b1_d = nc.dram_tensor("b1", [c.HF], FP32, kind="ExternalInput").ap()
    wout_d = nc.dram_tensor("wout", [c.KOUT, P, c.D], BF16, kind="ExternalInput").ap()
    bout_d = nc.dram_tensor("bout", [P, c.KD * c.BL], FP32, kind="ExternalInput").ap()
    y_d = nc.dram_tensor("y", [c.BL, c.D], FP32, kind="ExternalOutput").ap()

    with tile.TileContext(nc) as tc:
        with (
            tc.tile_pool(name="const", bufs=1) as const,
            tc.tile_pool(name="xT", bufs=4) as xT_pool,
            tc.tile_pool(name="h", bufs=4) as h_pool,
            tc.tile_pool(name="xn", bufs=3) as xn_pool,
            tc.tile_pool(name="e", bufs=10) as e_pool,
            tc.tile_pool(name="small", bufs=8) as small_pool,
            tc.tile_pool(name="ysb", bufs=1) as ysb_pool,
            tc.tile_pool(name="hps", bufs=3, space="PSUM") as hps_pool,
            tc.tile_pool(name="scps", bufs=4, space="PSUM") as scps_pool,
            tc.tile_pool(name="plps", bufs=1, space="PSUM") as plps_pool,
            tc.tile_pool(name="acc", bufs=2) as acc_pool,
        ):
            # ---- constants / weights ----
            # W1 as 4 column-quarter tiles: the first matmul group only
            # waits for quarter 0 (~1MB); the rest stream in behind it.
            QW = c.HF // 4
            QMC = c.MC // 4  # f-chunks per quarter
            w1qh = [
                const.tile([P, c.KD, QW], F8H, tag=f"w1qh{q}", name=f"w1qh{q}")
                for q in range(4)
            ]
            w1ql = [
                const.tile([P, c.KD, QW], F8L, tag=f"w1ql{q}", name=f"w1ql{q}")
                if W1TERMS >= 3 else None
                for q in range(4)
            ]

            def w1_piece(q, lo, eng=None):
                # one fp8 quarter (0.5MB, 512B innermost -> full DMA rate);
                # hi quarters stream on sync, lo on gpsimd, concurrently
                dst, src = (w1ql[q], w1l_d) if lo else (w1qh[q], w1h_d)
                (eng or nc.sync).dma_start(
                    dst[:],
                    src[:, :, ts(q, QW)].rearrange("k p f -> p k f"),
                )

            w1_piece(0, 0)
            w2_sb = const.tile([P, c.MC, c.H], BF16)
            b1_sb = const.tile([P, c.MC], FP32)
            v_sb = const.tile([P, c.BL, c.NCV], BF16)
            bout_sb = const.tile([P, c.KD * c.BL], FP32)
            # pooled rows at r = b*32 + h (32-aligned per batch for DVE)
            # pooled^T accumulates directly in [128 d, (dc, h)] tiles; the
            # out-projection picks per-(dc,h) batch columns from poolT_sb
            poolT_sb = const.tile([P, c.KD, c.BL, c.H], BF16)
            ones_f32 = const.tile([1, P], FP32)
            nc.gpsimd.memset(ones_f32[:], 1.0)
            wout_sb = const.tile([P, c.KOUT, c.D], BF16)

            pending = []  # deferred (other-block) op emitters

            def flush(limit=0):
                # keep `limit` closures queued: pool matmuls lag their
                # sub-block by a few positions so early ones never stall
                # the PE on the xn DMA
                while len(pending) > limit:
                    pending.pop(0)()

            # flat chunk schedule across batches; xT DMAs prefetch 2 ahead
            sched = []
            for b in range(c.BL):
                t0 = 0
                for bi, TB in enumerate(c.blocks(b)):
                    sched.append((b, bi, t0, TB))
                    t0 += TB
            xts = {}

            def ensure_xt(k):
                if k >= len(sched) or k in xts:
                    return
                b, bi, t0, TB = sched[k]
                xTh = xT_pool.tile([P, c.KD, 512], F8H, tag="xth", name=f"xTh{k % 4}")
                xTl = xT_pool.tile([P, c.KD, 512], F8L, tag="xtl", name=f"xTl{k % 4}")
                if k == 0:
                    # first xt tiles: hi on scalar, lo on gpsimd - both
                    # transfer concurrently with w1q0-hi on sync
                    nc.scalar.dma_start(
                        xTh[:, :, 0:TB],
                        xth_d[b, :, :, t0 : t0 + TB].rearrange("k p t -> p k t"),
                    )
                    nc.gpsimd.dma_start(
                        xTl[:, :, 0:TB],
                        xtl_d[b, :, :, t0 : t0 + TB].rearrange("k p t -> p k t"),
                    )
                    # small weights behind them on the scalar queue
                    nc.scalar.dma_start(
                        b1_sb[:], b1_d.rearrange("(c p) -> p c", p=P)
                    )
                    nc.scalar.dma_start(
                        w2_sb[:], w2_d.rearrange("c p h -> p c h")
                    )
                    nc.scalar.dma_start(
                        v_sb[:], v_d.rearrange("b p c -> p b c")
                    )
                    nc.scalar.dma_start(bout_sb[:], bout_d)
                    # rest of W1: hi quarters on sync, lo on gpsimd
                    if W1TERMS >= 3:
                        w1_piece(0, 1, nc.gpsimd)
                    for q in range(1, 4):
                        w1_piece(q, 0)
                        if W1TERMS >= 3:
                            w1_piece(q, 1, nc.gpsimd)
                else:
                    nc.sync.dma_start(
                        xTh[:, :, 0:TB],
                        xth_d[b, :, :, t0 : t0 + TB].rearrange("k p t -> p k t"),
                    )
                    nc.sync.dma_start(
                        xTl[:, :, 0:TB],
                        xtl_d[b, :, :, t0 : t0 + TB].rearrange("k p t -> p k t"),
                    )
                xts[k] = (xTh, xTl)

            kprev = -1
            for b in range(c.BL):
                # pl/z PSUM tiles are bufs=1: allocate only after the
                # previous batch's deferred consumers have been emitted
                # (first flush of this batch), so buffer-reuse tracking
                # sees ops in order.
                z_ps = acc_sb = None
                NCVb = -(-c.NVS[b] // P)
                for bi, TB in enumerate(c.blocks(b)):
                    kprev += 1
                    k = kprev
                    _, _, t0, _ = sched[k]
                    cn0 = t0 // P
                    first = k == 0 and TB == 512
                    ensure_xt(k)
                    ensure_xt(k + 1)
                    ensure_xt(k + 2)
                    ensure_xt(k + 3)
                    xTh, xTl = xts.pop(k)
                    # xn rides the (otherwise idle) gpsimd DMA queue so it
                    # never contends with the xt/wout stream on sync
                    xnt = xn_pool.tile([P, 4, c.D], BF16)
                    fullt = (TB // P) * P  # whole 128-token chunks
                    if fullt:
                        nc.gpsimd.dma_start(
                            xnt[:, 0 : TB // P, :],
                            xn_d[b, t0 : t0 + fullt, :].rearrange(
                                "(s p) d -> p s d", p=P
                            ),
                        )
                    if TB % P:
                        nc.gpsimd.dma_start(
                            xnt[0 : TB % P, TB // P, :],
                            xn_d[b, t0 + fullt : t0 + TB, :],
                        )
                    if b == 0 and bi == 1:
                        # prefetch the output projection during the middle
                        KQ = c.KOUT // 4
                        for q in range(4):
                            nc.sync.dma_start(
                                wout_sb[:, ts(q, KQ), :],
                                wout_d[ts(q, KQ)].rearrange("k p f -> p k f"),
                            )
                    # 128-token sub-blocks processed in PAIRS: the 3 fp8
                    # split terms accumulate per sub as sequential PSUM
                    # groups in one tile, then ONE tanh covers the pair
                    # (halving the Act engine's fixed access cost).
                    subs = -(-TB // P)
                    wid = [min(P, TB - s * P) for s in range(subs)]
                    groups = [
                        list(range(i, min(i + 4, subs)))
                        for i in range(0, subs, 4)
                    ]
                    for group in groups:
                        offs = {}
                        o = 0
                        for s in group:
                            offs[s] = o
                            o += wid[s]
                        wt = o  # total pair width
                        scs, prevq = {}, []
                        for s in group:
                            scs[s] = scps_pool.tile(
                                [P, 512], FP32, tag="sc_ps", name=f"sc{s}"
                            )

                        def dot(s, mc, h_sb):
                            nc.tensor.matmul(
                                scs[s][0 : wid[s], 0 : c.H],
                                h_sb[:, offs[s] : offs[s] + wid[s]],
                                w2_sb[:, mc, :],
                                start=(mc == 0),
                                stop=(mc == c.MC - 1),
                            )

                        for mc in range(c.MC):
                            h_ps = hps_pool.tile([P, 512], FP32, tag="h_ps")
                            wcol = (mc % QMC) * P
                            q = mc // QMC
                            terms = [(w1qh[q], xTh), (w1qh[q], xTl),
                                     (w1ql[q], xTh)][:W1TERMS]
                            for s in group:
                                w, off = wid[s], offs[s]
                                sp = s * P
                                for t, (lh, rh) in enumerate(terms):
                                    for pr in range(c.KD // 2):
                                        dc = 2 * pr
                                        nc.tensor.matmul(
                                            h_ps[:, off : off + w],
                                            lh[:, dc : dc + 2, wcol : wcol + P],
                                            rh[:, dc : dc + 2, sp : sp + w],
                                            start=(t == 0 and pr == 0),
                                            stop=(
                                                t == len(terms) - 1
                                                and pr == c.KD // 2 - 1
                                            ),
                                            perf_mode=DR,
                                        )
                            h_sb = h_pool.tile([P, 4 * P], BF16, tag="h_sb")
                            nc.scalar.activation(
                                h_sb[:, 0:wt], h_ps[:, 0:wt], AFT.Tanh,
                                bias=b1_sb[:, mc : mc + 1],
                                scale=1.0 / W1SCALE,
                            )
                            prevq.append((mc, h_sb))
                            if len(prevq) > 2:
                                pm, ph = prevq.pop(0)
                                for s in group:
                                    dot(s, pm, ph)
                            if mc == 2:
                                # drain fully in the kernel's last chunk so
                                # deferred pool work doesn't pile into the tail
                                flush(0 if k == len(sched) - 1 else 3)
                        while prevq:
                            pm, ph = prevq.pop(0)
                            for s in group:
                                dot(s, pm, ph)
                        if acc_sb is None:
                            # full flush: the previous batch's pool/finish
                            # closures must be emitted before the acc
                            # buffer rotates to this batch
                            flush()
                            acc_sb = acc_pool.tile(
                                [P, c.KD * c.H + c.H], FP32, tag="acc"
                            )
                        for s in group:
                            w = wid[s]
                            e_blk = e_pool.tile([P, c.H], BF16, tag="e_blk")
                            nc.scalar.activation(
                                e_blk[0:w, :], scs[s][0:w, 0 : c.H],
                                AFT.Exp, bias=0.0,
                            )

                            def mk_pool(b, s, w, cn, ncv, e_blk, xnt, acc_sb):
                                def go():
                                    # pooled^T chunk: [128 d, H] per dc -
                                    # free dim 4, nearly free on the PE -
                                    # plus Z[h] = v . e on partition 0;
                                    # DVE accumulates chunks across cn
                                    NA = c.KD * c.H
                                    plw = plps_pool.tile(
                                        [P, 512], FP32, tag="plw", name="plw"
                                    )
                                    for dc in range(c.KD):
                                        nc.tensor.matmul(
                                            plw[:, dc * c.H : (dc + 1) * c.H],
                                            xnt[0:w, s, ts(dc, P)],
                                            e_blk[0:w, :],
                                            start=True,
                                            stop=True,
                                        )
                                    nc.tensor.matmul(
                                        plw[0:1, NA : NA + c.H],
                                        v_sb[0:w, b, cn : cn + 1],
                                        e_blk[0:w, :],
                                        start=True,
                                        stop=True,
                                    )
                                    if cn == 0:
                                        nc.vector.tensor_copy(
                                            acc_sb[:, 0:NA], plw[:, 0:NA]
                                        )
                                        nc.vector.tensor_copy(
                                            acc_sb[0:1, NA : NA + c.H],
                                            plw[0:1, NA : NA + c.H],
                                        )
                                    else:
                                        nc.vector.tensor_add(
                                            acc_sb[:, 0:NA], acc_sb[:, 0:NA],
                                            plw[:, 0:NA],
                                        )
                                        nc.vector.tensor_add(
                                            acc_sb[0:1, NA : NA + c.H],
                                            acc_sb[0:1, NA : NA + c.H],
                                            plw[0:1, NA : NA + c.H],
                                        )
                                return go

                            pending.append(
                                mk_pool(b, s, w, cn0 + s, NCVb, e_blk, xnt,
                                        acc_sb)
                            )
                    t0 += TB

                def mk_finish(b, acc_sb):
                    def go():
                        NA = c.KD * c.H
                        rz = small_pool.tile([1, c.H], FP32, tag="rz")
                        nc.vector.reciprocal(rz[:], acc_sb[0:1, NA : NA + c.H])
                        # broadcast rz down the partitions via the PE, then
                        # scale acc into poolT_sb one head-column at a time
                        rzb_ps = scps_pool.tile(
                            [P, 512], FP32, tag="sc_ps", name="rzb"
                        )
                        nc.tensor.matmul(
                            rzb_ps[:, 0 : c.H], ones_f32[:], rz[:],
                            start=True, stop=True,
                        )
                        rzb = small_pool.tile([P, c.H], FP32, tag="rzb")
                        nc.vector.tensor_copy(rzb[:], rzb_ps[:, 0 : c.H])
                        accv = acc_sb[:, 0:NA].rearrange(
                            "p (k h) -> p h k", h=c.H
                        )
                        for hd in range(c.H):
                            nc.vector.tensor_scalar_mul(
                                poolT_sb[:, :, b, hd],
                                accv[:, hd, :],
                                rzb[:, hd : hd + 1],
                            )
                    return go

                pending.append(mk_finish(b, acc_sb))

            flush()
            # ---- output projection y = pooled @ Wout + bout ----
            # y^T chunks [128 d_out, BL]: free dim = BL = 4, so the 256
            # matmuls cost the PE almost nothing (vs 64 x 512-free rows
            # the straight orientation would charge). Groups sequential
            # per d_out chunk -> one PSUM region suffices.
            yt_ps = plps_pool.tile([P, 512], FP32, tag="plw", name="yt_ps")
            ytcols = yt_ps[:, 0 : c.BL * c.KD].rearrange(
                "p (b o) -> p o b", o=c.KD
            )
            for do in range(c.KD):
                for hd in range(c.H):
                    for dc in range(c.KD):
                        nc.tensor.matmul(
                            ytcols[:, do, :],
                            wout_sb[:, hd * c.KD + dc, ts(do, P)],
                            poolT_sb[:, dc, :, hd],
                            start=(hd == 0 and dc == 0),
                            stop=(hd == c.H - 1 and dc == c.KD - 1),
                        )
            # += bout (pre-laid as [P, KD*BL]), then a strided DMA writes
            # y directly in [BL, D] order - no transposes, no extra copy
            yt_sb = ysb_pool.tile([P, c.KD * c.BL], FP32)
            nc.vector.tensor_add(
                yt_sb[:], yt_ps[:, 0 : c.KD * c.BL], bout_sb[:]
            )
            nc.sync.dma_start(
                y_d.rearrange("b (o p) -> p b o", p=P),
                yt_sb[:].rearrange("p (b o) -> p b o", o=c.KD),
            )
    return nc


def make_in_maps(x, valid_mask, W1, b1, W2, b2, Wout, bout, n_cores, cfg,
                 order=None):
    """Host-side prep: compact valid tokens, shard over batch, cast/layout."""
    c = cfg
    bf16 = ml_dtypes.bfloat16
    e4 = ml_dtypes.float8_e4m3
    e5 = ml_dtypes.float8_e5m2
    B = x.shape[0]
    # W1 split into fp8 hi + e5m2 residual (used in DoubleRow mode),
    # pre-scaled so its sigma sits well inside e4m3's normal range; the
    # tanh's scale undoes it exactly
    w1f = np.ascontiguousarray(
        W1.transpose(1, 0, 2).reshape(c.KD, P, c.HF).astype(np.float32)
        * np.float32(W1SCALE)
    )
    w1h_l = np.ascontiguousarray(w1f.astype(e4))
    w1l_l = np.ascontiguousarray(
        (w1f - w1h_l.astype(np.float32)).astype(e5)
    )
    w2f = W2.reshape(c.HF).astype(np.float32)
    w2_l = np.zeros((c.MC, P, c.H), np.float32)
    FC = c.MC // c.H  # f-chunks per head
    for mc in range(c.MC):
        w2_l[mc, :, mc // FC] = w2f[mc * P : (mc + 1) * P]
    w2_l = np.ascontiguousarray(w2_l.astype(bf16))
    b1_l = np.ascontiguousarray(b1.reshape(c.HF).astype(np.float32))
    wout_l = np.ascontiguousarray(Wout.reshape(c.KOUT, P, c.D).astype(bf16))
    # bout pre-laid for the y^T tail: bout_l[p, b*KD + do] = bout[do*128+p]
    bout_l = np.ascontiguousarray(
        np.tile(bout.astype(np.float32).reshape(c.KD, P).T, (1, c.BL))
    )
    # b2 is a per-row constant under the softmax -> it cancels; drop it.
    if order is None:
        order = np.arange(B)
    xc32 = np.zeros((B, c.NV, c.D), np.float32)
    v = np.zeros((B, c.NCV * P), np.float32)
    for gb in range(B):
        idx = np.flatnonzero(valid_mask[gb])[: c.NV]
        xc32[gb, : len(idx)] = x[gb, idx]
        v[gb, : len(idx)] = 1.0
    v_l = np.ascontiguousarray(
        v.reshape(B, c.NCV, P).transpose(0, 2, 1).astype(bf16)
    )
    xc = xc32.astype(bf16)  # pooling operand
    # x^T split into fp8 hi + e5m2 residual for the score matmul
    xt32 = np.ascontiguousarray(xc32.transpose(0, 2, 1)).reshape(
        B, c.KD, P, c.NV
    )
    xth_all = np.ascontiguousarray(xt32.astype(e4))
    xtl_all = np.ascontiguousarray(
        (xt32 - xth_all.astype(np.float32)).astype(e5)
    )
    in_maps = []
    for core in range(n_cores):
        sel = [order[i * n_cores + core] for i in range(c.BL)]
        in_maps.append(
            {
                "xth": np.ascontiguousarray(xth_all[sel]),
                "xtl": np.ascontiguousarray(xtl_all[sel]),
                "xn": np.ascontiguousarray(xc[sel]),
                "v": np.ascontiguousarray(v_l[sel]),
                "w1h": w1h_l,
                "w1l": w1l_l,
                "w2": w2_l,
                "b1": b1_l,
                "wout": wout_l,
                "bout": bout_l,
            }
        )
    return in_maps


_cached = {}
last_results = None


def kernel(x, valid_mask, W1, b1, W2, b2, Wout, bout, trace=False):
    global last_results
    x, valid_mask, W1, b1, W2, b2, Wout, bout = (
        np.asarray(a)
        for a in (x, valid_mask, W1, b1, W2, b2, Wout, bout)
    )
    B = x.shape[0]
    n_cores = 8
    NVS, order = choose_slots(valid_mask, n_cores, B // n_cores)
    cfg = Cfg(BL=B // n_cores, NVS=NVS)
    key = (B, NVS)
    if key not in _cached:
        nc = bacc.Bacc("TRN2", target_bir_lowering=False, debug=False)
        build_kernel(nc, cfg)
        nc.compile()
        _cached[key] = nc
    in_maps = make_in_maps(
        x, valid_mask, W1, b1, W2, b2, Wout, bout, n_cores, cfg, order
    )
    res = run_bass_kernel_spmd(
        _cached[key], in_maps, core_ids=list(range(n_cores)), trace=trace
    )
    last_results = res
    y = np.empty((B, x.shape[2]), np.float32)
    for core in range(n_cores):
        yc = np.asarray(res.results[core]["y"], np.float32)
        for i in range(cfg.BL):
            y[order[i * n_cores + core]] = yc[i]
    return y


# revision 103
# speedup vs baseline: 2.0056x; 1.1073x over previous
"""AttentionPool Trainium2 Bass kernel (valid-token compaction).

Reference computation (per batch b):
    h      = tanh(x @ W1 + b1)          # [N, H*F]   (big matmul, bf16 on PE)
    scores = h @ W2 + b2                # [N, H]
    scores = where(mask, scores, -1e9)
    w      = softmax(scores, axis=N)    # per head
    pooled = w.T @ x                    # [H, D]
    y      = concat_h(pooled) @ Wout + bout   # [D]

Key structural ideas:
  - Invalid tokens get softmax weight 0 and contribute nothing to the
    output, so the host compacts each batch's valid tokens (~1024 of 2048
    at p=0.5) into a contiguous buffer and the big x@W1 matmul runs only
    on those (~2x less PE work). Batches are sorted by valid count and
    assigned to per-slot lengths NVS (exact, partial trailing 128-token
    chunks are fine), so every core does the same near-minimal work.
  - Padding inside a slot carries x=0: it cannot pollute the pooling sum,
    and the softmax denominator is computed as a validity-weighted PE
    reduction Z = sum_t v_t e_t (v in {1,0}), so no -1e9 mask tensor or
    max-shift exists anywhere. b2 cancels under softmax and is dropped.
  - All small-output reductions are shaped so the PE streams tiny free
    dims (matmul cost ~ out free size): the score dot emits [tok, H=4]
    tiles, pooling emits pooled^T [128d, H=4] tiles per (chunk, dc) that
    the DVE accumulates across chunks, and the output projection emits
    y^T [128d, B=4] columns. Everything lands pre-transposed for its
    consumer; the only PE transposes left are none.
  - The exp(scores) tiles are already token-major = the pooling lhsT.

Sharding: data-parallel over batch B=32 across 8 cores (4 batches/core,
count-sorted slot assignment, results un-permuted on the host). Weights
replicated. Matmuls bf16 (fp32 PSUM); softmax fp32 on the Act engine
(the exp_and_others act table holds both tanh and exp - no table loads).
|scores| <= ||W2||_1 ~ 18, so exp is safe without max-shift.

Pipelining (keeps the PE dense, which also keeps its p-state ramped):
  - per f-chunk mc: the 8 k-matmuls of chunk mc+1..mc+2 are issued before
    the score dot of chunk mc (depth-2 queue), hiding the tanh latency.
  - a sub-block's pooling/Z matmuls are deferred 3 sub-blocks down the
    instruction stream, when exp and the xn DMA have long finished.
  - W1 streams in 256-column half-quarters on two DMA queues (sync +
    gpsimd) paced to the first chunk's interleaved sub-pair consumption;
    xT prefetches 2 chunks ahead; xn rides the gpsimd queue.
  - narrow trailing sub-blocks are interleave-paired with a full one so
    PSUM h-buffer recycling never waits on the tanh pipeline.
"""

import numpy as np
import ml_dtypes

import concourse.bass as bass
import concourse.mybir as mybir
import concourse.tile as tile
from concourse import bacc
from concourse.bass import ts
from concourse.bass_utils import run_bass_kernel_spmd
BF16 = mybir.dt.bfloat16
FP32 = mybir.dt.float32
F8H = mybir.dt.float8e4
F8L = mybir.dt.float8e5
DR = mybir.MatmulPerfMode.DoubleRow
AFT = mybir.ActivationFunctionType

P = 128
# fp8 split of the score matmul: 2 terms = xh.Wh + xl.Wh (W1 pre-scaled
# by W1SCALE, exactly undone by the tanh's scale; measured end-to-end rel
# err 1.35e-2 vs the 2e-2 gate). Set to 3 to add the xh.Wl term
# (rel 1.8e-3) at 1.5x the PE cost of 2 terms.
W1TERMS = 2
W1SCALE = 64.0
# number of k-chunk pairs (of KD/2=4) whose x_lo correction is dropped
# (1 of 4: measured full-batch rel 1.6e-2 vs the 2e-2 gate; 2 would save
# another 12.5% PE but projects to ~1.9e-2 - too close)
XLO_SKIP = 1


class Cfg:
    def __init__(self, BL=4, NVS=(1152,) * 4, D=1024, H=4, F=512):
        # NVS: per-batch-slot padded valid-token count (batches are
        # assigned to slots sorted by count, so later slots can be shorter)
        self.BL, self.NVS, self.D, self.H, self.F = BL, tuple(NVS), D, H, F
        assert len(self.NVS) == BL
        self.NV = max(self.NVS)   # shipped tensor size
        self.HF = H * F
        self.KD = D // P          # k-chunks of D
        self.MC = self.HF // P    # f-chunks of H*F
        self.NCV = -(-self.NV // P)  # token chunks (max slot, ceil)
        self.KOUT = (H * D) // P  # k-chunks of the output projection

    def blocks(self, b):
        """DMA token blocks for batch slot b: 512-wide + remainder. A
        short (<=128) remainder is merged with 128 tokens borrowed from
        the previous block so the final chunk has two sub-blocks - the
        narrow one is interleave-paired with a full one to keep the PSUM
        buffer-recycle window wider than the tanh pipeline latency."""
        nv = self.NVS[b]
        out = [512] * (nv // 512)
        r = nv % 512
        if r:
            if r <= P and out:
                out[-1] -= P
                r += P
            out.append(r)
        return out


def choose_slots(valid_mask: np.ndarray, n_cores=8, BL=4):
    """Sort batches by valid count; slot i takes ranks [i*n_cores,
    (i+1)*n_cores) so each slot's NV covers its 8 batches exactly
    (partial trailing token chunks are fine - tokens ride free/K dims).
    Returns (NVS, order) with order[i*n_cores + c] = original batch
    index processed by core c in slot i."""
    cnt = np.asarray(valid_mask).sum(axis=1)
    order = np.argsort(-cnt, kind="stable")
    NVS = []
    for i in range(BL):
        grp = cnt[order[i * n_cores : (i + 1) * n_cores]]
        NVS.append(max(2, int(grp.max())))
    return tuple(NVS), order


def build_kernel(nc: bass.Bass, cfg: Cfg, reps: int = 1):
    c = cfg
    # x^T and W1 ship as fp8 hi (e4m3) + residual lo (e5m2); the h matmul
    # runs 3 split terms (xh.Wh + xl.Wh + xh.Wl) in DoubleRow mode, each
    # instruction contracting TWO 128-deep k-chunks at 0.5 cycles/row -
    # 0.75x the bf16 PE cost at better-than-bf16 accuracy (the dropped
    # xl.Wl term is ~2^-8 relative).
    xth_d = nc.dram_tensor("xth", [c.BL, c.KD, P, c.NV], F8H, kind="ExternalInput").ap()
    xtl_d = nc.dram_tensor("xtl", [c.BL, c.KD, P, c.NV], F8L, kind="ExternalInput").ap()
    xn_d = nc.dram_tensor("xn", [c.BL, c.NV, c.D], BF16, kind="ExternalInput").ap()
    v_d = nc.dram_tensor("v", [c.BL, P, c.NCV], BF16, kind="ExternalInput").ap()
    w1h_d = nc.dram_tensor("w1h", [c.KD, P, c.HF], F8H, kind="ExternalInput").ap()
    w1l_d = nc.dram_tensor("w1l", [c.KD, P, c.HF], F8L, kind="ExternalInput").ap()
    w2_d = nc.dram_tensor("w2", [c.MC, P, c.H], BF16, kind="ExternalInput").ap()
    b1_d = nc.dram_tensor("b1", [c.HF], FP32, kind="ExternalInput").ap()
    wout_d = nc.dram_tensor("wout", [c.KOUT, P, c.D], BF16, kind="ExternalInput").ap()
    bout_d = nc.dram_tensor("bout", [P, c.KD * c.BL], FP32, kind="ExternalInput").ap()
    y_d = nc.dram_tensor("y", [c.BL, c.D], FP32, kind="ExternalOutput").ap()

    with tile.TileContext(nc) as tc:
        with (
            tc.tile_pool(name="const", bufs=1) as const,
            tc.tile_pool(name="xT", bufs=4) as xT_pool,
            tc.tile_pool(name="h", bufs=4) as h_pool,
            tc.tile_pool(name="xn", bufs=3) as xn_pool,
            tc.tile_pool(name="e", bufs=10) as e_pool,
            tc.tile_pool(name="small", bufs=8) as small_pool,
            tc.tile_pool(name="ysb", bufs=1) as ysb_pool,
            tc.tile_pool(name="hps", bufs=3, space="PSUM") as hps_pool,
            tc.tile_pool(name="scps", bufs=4, space="PSUM") as scps_pool,
            tc.tile_pool(name="plps", bufs=1, space="PSUM") as plps_pool,
            tc.tile_pool(name="acc", bufs=2) as acc_pool,
        ):
            # ---- constants / weights ----
            # W1 as 4 column-quarter tiles: the first matmul group only
            # waits for quarter 0 (~1MB); the rest stream in behind it.
            QW = c.HF // 4
            QMC = c.MC // 4  # f-chunks per quarter
            w1qh = [
                const.tile([P, c.KD, QW], F8H, tag=f"w1qh{q}", name=f"w1qh{q}")
                for q in range(4)
            ]
            w1ql = [
                const.tile([P, c.KD, QW], F8L, tag=f"w1ql{q}", name=f"w1ql{q}")
                if W1TERMS >= 3 else None
                for q in range(4)
            ]

            def w1_piece(q, lo, eng=None):
                # one fp8 quarter (0.5MB, 512B innermost -> full DMA rate);
                # hi quarters stream on sync, lo on gpsimd, concurrently
                dst, src = (w1ql[q], w1l_d) if lo else (w1qh[q], w1h_d)
                (eng or nc.sync).dma_start(
                    dst[:],
                    src[:, :, ts(q, QW)].rearrange("k p f -> p k f"),
                )

            w1_piece(0, 0)
            w2_sb = const.tile([P, c.MC, c.H], BF16)
            b1_sb = const.tile([P, c.MC], FP32)
            v_sb = const.tile([P, c.BL, c.NCV], BF16)
            bout_sb = const.tile([P, c.KD * c.BL], FP32)
            # pooled rows at r = b*32 + h (32-aligned per batch for DVE)
            # pooled^T accumulates directly in [128 d, (dc, h)] tiles; the
            # out-projection picks per-(dc,h) batch columns from poolT_sb
            poolT_sb = const.tile([P, c.KD, c.BL, c.H], BF16)
            ones_f32 = const.tile([1, P], FP32)
            nc.gpsimd.memset(ones_f32[:], 1.0)
            wout_sb = const.tile([P, c.KOUT, c.D], BF16)

            pending = []  # deferred (other-block) op emitters

            def flush(limit=0):
                # keep `limit` closures queued: pool matmuls lag their
                # sub-block by a few positions so early ones never stall
                # the PE on the xn DMA
                while len(pending) > limit:
                    pending.pop(0)()

            # flat chunk schedule across batches; xT DMAs prefetch 2 ahead
            sched = []
            for b in range(c.BL):
                t0 = 0
                for bi, TB in enumerate(c.blocks(b)):
                    sched.append((b, bi, t0, TB))
                    t0 += TB
            xts = {}

            def ensure_xt(k):
                if k >= len(sched) or k in xts:
                    return
                b, bi, t0, TB = sched[k]
                xTh = xT_pool.tile([P, c.KD, 512], F8H, tag="xth", name=f"xTh{k % 4}")
                xTl = xT_pool.tile([P, c.KD, 512], F8L, tag="xtl", name=f"xTl{k % 4}")
                if k == 0:
                    # first xt tiles: hi on scalar, lo on gpsimd - both
                    # transfer concurrently with w1q0-hi on sync
                    nc.scalar.dma_start(
                        xTh[:, :, 0:TB],
                        xth_d[b, :, :, t0 : t0 + TB].rearrange("k p t -> p k t"),
                    )
                    nc.gpsimd.dma_start(
                        xTl[:, :, 0:TB],
                        xtl_d[b, :, :, t0 : t0 + TB].rearrange("k p t -> p k t"),
                    )
                    # small weights behind them on the scalar queue
                    nc.scalar.dma_start(
                        b1_sb[:], b1_d.rearrange("(c p) -> p c", p=P)
                    )
                    nc.scalar.dma_start(
                        w2_sb[:], w2_d.rearrange("c p h -> p c h")
                    )
                    nc.scalar.dma_start(
                        v_sb[:], v_d.rearrange("b p c -> p b c")
                    )
                    nc.scalar.dma_start(bout_sb[:], bout_d)
                    # rest of W1: hi quarters on sync, lo on gpsimd
                    if W1TERMS >= 3:
                        w1_piece(0, 1, nc.gpsimd)
                    for q in range(1, 4):
                        w1_piece(q, 0)
                        if W1TERMS >= 3:
                            w1_piece(q, 1, nc.gpsimd)
                else:
                    nc.sync.dma_start(
                        xTh[:, :, 0:TB],
                        xth_d[b, :, :, t0 : t0 + TB].rearrange("k p t -> p k t"),
                    )
                    nc.sync.dma_start(
                        xTl[:, :, 0:TB],
                        xtl_d[b, :, :, t0 : t0 + TB].rearrange("k p t -> p k t"),
                    )
                xts[k] = (xTh, xTl)

            kprev = -1
            for b in range(c.BL):
                # pl/z PSUM tiles are bufs=1: allocate only after the
                # previous batch's deferred consumers have been emitted
                # (first flush of this batch), so buffer-reuse tracking
                # sees ops in order.
                z_ps = acc_sb = None
                NCVb = -(-c.NVS[b] // P)
                for bi, TB in enumerate(c.blocks(b)):
                    kprev += 1
                    k = kprev
                    _, _, t0, _ = sched[k]
                    cn0 = t0 // P
                    first = k == 0 and TB == 512
                    ensure_xt(k)
                    ensure_xt(k + 1)
                    ensure_xt(k + 2)
                    ensure_xt(k + 3)
                    xTh, xTl = xts.pop(k)
                    # xn rides the (otherwise idle) gpsimd DMA queue so it
                    # never contends with the xt/wout stream on sync
                    xnt = xn_pool.tile([P, 4, c.D], BF16)
                    fullt = (TB // P) * P  # whole 128-token chunks
                    if fullt:
                        nc.gpsimd.dma_start(
                            xnt[:, 0 : TB // P, :],
                            xn_d[b, t0 : t0 + fullt, :].rearrange(
                                "(s p) d -> p s d", p=P
                            ),
                        )
                    if TB % P:
                        nc.gpsimd.dma_start(
                            xnt[0 : TB % P, TB // P, :],
                            xn_d[b, t0 + fullt : t0 + TB, :],
                        )
                    if b == 0 and bi == 1:
                        # prefetch the output projection during the middle
                        KQ = c.KOUT // 4
                        for q in range(4):
                            nc.sync.dma_start(
                                wout_sb[:, ts(q, KQ), :],
                                wout_d[ts(q, KQ)].rearrange("k p f -> p k f"),
                            )
                    # 128-token sub-blocks processed in PAIRS: the 3 fp8
                    # split terms accumulate per sub as sequential PSUM
                    # groups in one tile, then ONE tanh covers the pair
                    # (halving the Act engine's fixed access cost).
                    subs = -(-TB // P)
                    wid = [min(P, TB - s * P) for s in range(subs)]
                    groups = [
                        list(range(i, min(i + 4, subs)))
                        for i in range(0, subs, 4)
                    ]
                    for group in groups:
                        offs = {}
                        o = 0
                        for s in group:
                            offs[s] = o
                            o += wid[s]
                        wt = o  # total pair width
                        scs, prevq = {}, []
                        for s in group:
                            scs[s] = scps_pool.tile(
                                [P, 512], FP32, tag="sc_ps", name=f"sc{s}"
                            )

                        def dot(s, mc, h_sb):
                            nc.tensor.matmul(
                                scs[s][0 : wid[s], 0 : c.H],
                                h_sb[:, offs[s] : offs[s] + wid[s]],
                                w2_sb[:, mc, :],
                                start=(mc == 0),
                                stop=(mc == c.MC - 1),
                            )

                        for mc in range(c.MC):
                            h_ps = hps_pool.tile([P, 512], FP32, tag="h_ps")
                            wcol = (mc % QMC) * P
                            q = mc // QMC
                            terms = [(w1qh[q], xTh), (w1qh[q], xTl),
                                     (w1ql[q], xTh)][:W1TERMS]
                            for s in group:
                                w, off = wid[s], offs[s]
                                sp = s * P
                                for t, (lh, rh) in enumerate(terms):
                                    for pr in range(c.KD // 2):
                                        # the x_lo correction is skipped
                                        # for the first XLO_SKIP k-pairs:
                                        # adds x-quant error on 1/4 of the
                                        # contraction depth (measured rel
                                        # stays under the gate) for 12.5%
                                        # less PE work
                                        if t == 1 and pr < XLO_SKIP:
                                            continue
                                        dc = 2 * pr
                                        nc.tensor.matmul(
                                            h_ps[:, off : off + w],
                                            lh[:, dc : dc + 2, wcol : wcol + P],
                                            rh[:, dc : dc + 2, sp : sp + w],
                                            start=(t == 0 and pr == 0),
                                            stop=(
                                                t == len(terms) - 1
                                                and pr == c.KD // 2 - 1
                                            ),
                                            perf_mode=DR,
                                        )
                            h_sb = h_pool.tile([P, 4 * P], BF16, tag="h_sb")
                            nc.scalar.activation(
                                h_sb[:, 0:wt], h_ps[:, 0:wt], AFT.Tanh,
                                bias=b1_sb[:, mc : mc + 1],
                                scale=1.0 / W1SCALE,
                            )
                            prevq.append((mc, h_sb))
                            if len(prevq) > 2:
                                pm, ph = prevq.pop(0)
                                for s in group:
                                    dot(s, pm, ph)
                            if mc == 2:
                                # drain fully in the kernel's last chunk so
                                # deferred pool work doesn't pile into the tail
                                flush(0 if k == len(sched) - 1 else 3)
                        while prevq:
                            pm, ph = prevq.pop(0)
                            for s in group:
                                dot(s, pm, ph)
                        if acc_sb is None:
                            # full flush: the previous batch's pool/finish
                            # closures must be emitted before the acc
                            # buffer rotates to this batch
                            flush()
                            acc_sb = acc_pool.tile(
                                [P, c.KD * c.H + c.H], FP32, tag="acc"
                            )
                        for s in group:
                            w = wid[s]
                            e_blk = e_pool.tile([P, c.H], BF16, tag="e_blk")
                            nc.scalar.activation(
                                e_blk[0:w, :], scs[s][0:w, 0 : c.H],
                                AFT.Exp, bias=0.0,
                            )

                            def mk_pool(b, s, w, cn, ncv, e_blk, xnt, acc_sb):
                                def go():
                                    # pooled^T chunk: [128 d, H] per dc -
                                    # free dim 4, nearly free on the PE -
                                    # plus Z[h] = v . e on partition 0;
                                    # DVE accumulates chunks across cn
                                    NA = c.KD * c.H
                                    plw = plps_pool.tile(
                                        [P, 512], FP32, tag="plw", name="plw"
                                    )
                                    for dc in range(c.KD):
                                        nc.tensor.matmul(
                                            plw[:, dc * c.H : (dc + 1) * c.H],
                                            xnt[0:w, s, ts(dc, P)],
                                            e_blk[0:w, :],
                                            start=True,
                                            stop=True,
                                        )
                                    nc.tensor.matmul(
                                        plw[0:1, NA : NA + c.H],
                                        v_sb[0:w, b, cn : cn + 1],
                                        e_blk[0:w, :],
                                        start=True,
                                        stop=True,
                                    )
                                    if cn == 0:
                                        nc.vector.tensor_copy(
                                            acc_sb[:, 0:NA], plw[:, 0:NA]
                                        )
                                        nc.vector.tensor_copy(
                                            acc_sb[0:1, NA : NA + c.H],
                                            plw[0:1, NA : NA + c.H],
                                        )
                                    else:
                                        nc.vector.tensor_add(
                                            acc_sb[:, 0:NA], acc_sb[:, 0:NA],
                                            plw[:, 0:NA],
                                        )
                                        nc.vector.tensor_add(
                                            acc_sb[0:1, NA : NA + c.H],
                                            acc_sb[0:1, NA : NA + c.H],
                                            plw[0:1, NA : NA + c.H],
                                        )
                                return go

                            pending.append(
                                mk_pool(b, s, w, cn0 + s, NCVb, e_blk, xnt,
                                        acc_sb)
                            )
                    t0 += TB

                def mk_finish(b, acc_sb):
                    def go():
                        NA = c.KD * c.H
                        rz = small_pool.tile([1, c.H], FP32, tag="rz")
                        nc.vector.reciprocal(rz[:], acc_sb[0:1, NA : NA + c.H])
                        # broadcast rz down the partitions via the PE, then
                        # scale acc into poolT_sb one head-column at a time
                        rzb_ps = scps_pool.tile(
                            [P, 512], FP32, tag="sc_ps", name="rzb"
                        )
                        nc.tensor.matmul(
                            rzb_ps[:, 0 : c.H], ones_f32[:], rz[:],
                            start=True, stop=True,
                        )
                        rzb = small_pool.tile([P, c.H], FP32, tag="rzb")
                        nc.vector.tensor_copy(rzb[:], rzb_ps[:, 0 : c.H])
                        accv = acc_sb[:, 0:NA].rearrange(
                            "p (k h) -> p h k", h=c.H
                        )
                        for hd in range(c.H):
                            nc.vector.tensor_scalar_mul(
                                poolT_sb[:, :, b, hd],
                                accv[:, hd, :],
                                rzb[:, hd : hd + 1],
                            )
                    return go

                pending.append(mk_finish(b, acc_sb))

            flush()
            # ---- output projection y = pooled @ Wout + bout ----
            # y^T chunks [128 d_out, BL]: free dim = BL = 4, so the 256
            # matmuls cost the PE almost nothing (vs 64 x 512-free rows
            # the straight orientation would charge). Groups sequential
            # per d_out chunk -> one PSUM region suffices.
            yt_ps = plps_pool.tile([P, 512], FP32, tag="plw", name="yt_ps")
            ytcols = yt_ps[:, 0 : c.BL * c.KD].rearrange(
                "p (b o) -> p o b", o=c.KD
            )
            for do in range(c.KD):
                for hd in range(c.H):
                    for dc in range(c.KD):
                        nc.tensor.matmul(
                            ytcols[:, do, :],
                            wout_sb[:, hd * c.KD + dc, ts(do, P)],
                            poolT_sb[:, dc, :, hd],
                            start=(hd == 0 and dc == 0),
                            stop=(hd == c.H - 1 and dc == c.KD - 1),
                        )
            # += bout (pre-laid as [P, KD*BL]), then a strided DMA writes
            # y directly in [BL, D] order - no transposes, no extra copy
            yt_sb = ysb_pool.tile([P, c.KD * c.BL], FP32)
            nc.vector.tensor_add(
                yt_sb[:], yt_ps[:, 0 : c.KD * c.BL], bout_sb[:]
            )
            nc.sync.dma_start(
                y_d.rearrange("b (o p) -> p b o", p=P),
                yt_sb[:].rearrange("p (b o) -> p b o", o=c.KD),
            )
    return nc


def make_in_maps(x, valid_mask, W1, b1, W2, b2, Wout, bout, n_cores, cfg,
                 order=None):
    """Host-side prep: compact valid tokens, shard over batch, cast/layout."""
    c = cfg
    bf16 = ml_dtypes.bfloat16
    e4 = ml_dtypes.float8_e4m3
    e5 = ml_dtypes.float8_e5m2
    B = x.shape[0]
    # W1 split into fp8 hi + e5m2 residual (used in DoubleRow mode),
    # pre-scaled so its sigma sits well inside e4m3's normal range; the
    # tanh's scale undoes it exactly
    w1f = np.ascontiguousarray(
        W1.transpose(1, 0, 2).reshape(c.KD, P, c.HF).astype(np.float32)
        * np.float32(W1SCALE)
    )
    w1h_l = np.ascontiguousarray(w1f.astype(e4))
    w1l_l = np.ascontiguousarray(
        (w1f - w1h_l.astype(np.float32)).astype(e5)
    )
    w2f = W2.reshape(c.HF).astype(np.float32)
    w2_l = np.zeros((c.MC, P, c.H), np.float32)
    FC = c.MC // c.H  # f-chunks per head
    for mc in range(c.MC):
        w2_l[mc, :, mc // FC] = w2f[mc * P : (mc + 1) * P]
    w2_l = np.ascontiguousarray(w2_l.astype(bf16))
    b1_l = np.ascontiguousarray(b1.reshape(c.HF).astype(np.float32))
    wout_l = np.ascontiguousarray(Wout.reshape(c.KOUT, P, c.D).astype(bf16))
    # bout pre-laid for the y^T tail: bout_l[p, b*KD + do] = bout[do*128+p]
    bout_l = np.ascontiguousarray(
        np.tile(bout.astype(np.float32).reshape(c.KD, P).T, (1, c.BL))
    )
    # b2 is a per-row constant under the softmax -> it cancels; drop it.
    if order is None:
        order = np.arange(B)
    xc32 = np.zeros((B, c.NV, c.D), np.float32)
    v = np.zeros((B, c.NCV * P), np.float32)
    for gb in range(B):
        idx = np.flatnonzero(valid_mask[gb])[: c.NV]
        xc32[gb, : len(idx)] = x[gb, idx]
        v[gb, : len(idx)] = 1.0
    v_l = np.ascontiguousarray(
        v.reshape(B, c.NCV, P).transpose(0, 2, 1).astype(bf16)
    )
    xc = xc32.astype(bf16)  # pooling operand
    # x^T split into fp8 hi + e5m2 residual for the score matmul
    xt32 = np.ascontiguousarray(xc32.transpose(0, 2, 1)).reshape(
        B, c.KD, P, c.NV
    )
    xth_all = np.ascontiguousarray(xt32.astype(e4))
    xtl_all = np.ascontiguousarray(
        (xt32 - xth_all.astype(np.float32)).astype(e5)
    )
    in_maps = []
    for core in range(n_cores):
        sel = [order[i * n_cores + core] for i in range(c.BL)]
        in_maps.append(
            {
                "xth": np.ascontiguousarray(xth_all[sel]),
                "xtl": np.ascontiguousarray(xtl_all[sel]),
                "xn": np.ascontiguousarray(xc[sel]),
                "v": np.ascontiguousarray(v_l[sel]),
                "w1h": w1h_l,
                "w1l": w1l_l,
                "w2": w2_l,
                "b1": b1_l,
                "wout": wout_l,
                "bout": bout_l,
            }
        )
    return in_maps


_cached = {}
last_results = None


def kernel(x, valid_mask, W1, b1, W2, b2, Wout, bout, trace=False):
    global last_results
    x, valid_mask, W1, b1, W2, b2, Wout, bout = (
        np.asarray(a)
        for a in (x, valid_mask, W1, b1, W2, b2, Wout, bout)
    )
    B = x.shape[0]
    n_cores = 8
    NVS, order = choose_slots(valid_mask, n_cores, B // n_cores)
    cfg = Cfg(BL=B // n_cores, NVS=NVS)
    key = (B, NVS)
    if key not in _cached:
        nc = bacc.Bacc("TRN2", target_bir_lowering=False, debug=False)
        build_kernel(nc, cfg)
        nc.compile()
        _cached[key] = nc
    in_maps = make_in_maps(
        x, valid_mask, W1, b1, W2, b2, Wout, bout, n_cores, cfg, order
    )
    res = run_bass_kernel_spmd(
        _cached[key], in_maps, core_ids=list(range(n_cores)), trace=trace
    )
    last_results = res
    y = np.empty((B, x.shape[2]), np.float32)
    for core in range(n_cores):
        yc = np.asarray(res.results[core]["y"], np.float32)
        for i in range(cfg.BL):
            y[order[i * n_cores + core]] = yc[i]
    return y


# revision 104
# speedup vs baseline: 2.2072x; 1.1005x over previous
"""AttentionPool Trainium2 Bass kernel (valid-token compaction).

Reference computation (per batch b):
    h      = tanh(x @ W1 + b1)          # [N, H*F]   (big matmul, bf16 on PE)
    scores = h @ W2 + b2                # [N, H]
    scores = where(mask, scores, -1e9)
    w      = softmax(scores, axis=N)    # per head
    pooled = w.T @ x                    # [H, D]
    y      = concat_h(pooled) @ Wout + bout   # [D]

Key structural ideas:
  - Invalid tokens get softmax weight 0 and contribute nothing to the
    output, so the host compacts each batch's valid tokens (~1024 of 2048
    at p=0.5) into a contiguous buffer and the big x@W1 matmul runs only
    on those (~2x less PE work). Batches are sorted by valid count and
    assigned to per-slot lengths NVS (exact, partial trailing 128-token
    chunks are fine), so every core does the same near-minimal work.
  - Padding inside a slot carries x=0: it cannot pollute the pooling sum,
    and the softmax denominator is computed as a validity-weighted PE
    reduction Z = sum_t v_t e_t (v in {1,0}), so no -1e9 mask tensor or
    max-shift exists anywhere. b2 cancels under softmax and is dropped.
  - All small-output reductions are shaped so the PE streams tiny free
    dims (matmul cost ~ out free size): the score dot emits [tok, H=4]
    tiles, pooling emits pooled^T [128d, H=4] tiles per (chunk, dc) that
    the DVE accumulates across chunks, and the output projection emits
    y^T [128d, B=4] columns. Everything lands pre-transposed for its
    consumer; the only PE transposes left are none.
  - The exp(scores) tiles are already token-major = the pooling lhsT.

Sharding: data-parallel over batch B=32 across 8 cores (4 batches/core,
count-sorted slot assignment, results un-permuted on the host). Weights
replicated. Matmuls bf16 (fp32 PSUM); softmax fp32 on the Act engine
(the exp_and_others act table holds both tanh and exp - no table loads).
|scores| <= ||W2||_1 ~ 18, so exp is safe without max-shift.

Pipelining (keeps the PE dense, which also keeps its p-state ramped):
  - per f-chunk mc: the 8 k-matmuls of chunk mc+1..mc+2 are issued before
    the score dot of chunk mc (depth-2 queue), hiding the tanh latency.
  - a sub-block's pooling/Z matmuls are deferred 3 sub-blocks down the
    instruction stream, when exp and the xn DMA have long finished.
  - W1 streams in 256-column half-quarters on two DMA queues (sync +
    gpsimd) paced to the first chunk's interleaved sub-pair consumption;
    xT prefetches 2 chunks ahead; xn rides the gpsimd queue.
  - narrow trailing sub-blocks are interleave-paired with a full one so
    PSUM h-buffer recycling never waits on the tanh pipeline.
"""

import numpy as np
import ml_dtypes

import concourse.bass as bass
import concourse.mybir as mybir
import concourse.tile as tile
from concourse import bacc
from concourse.bass import ts
from concourse.bass_utils import run_bass_kernel_spmd
BF16 = mybir.dt.bfloat16
FP32 = mybir.dt.float32
F8H = mybir.dt.float8e4
F8L = mybir.dt.float8e5
DR = mybir.MatmulPerfMode.DoubleRow
AFT = mybir.ActivationFunctionType

P = 128
# fp8 split of the score matmul: 2 terms = xh.Wh + xl.Wh (W1 pre-scaled
# by W1SCALE, exactly undone by the tanh's scale; measured end-to-end rel
# err 1.35e-2 vs the 2e-2 gate). Set to 3 to add the xh.Wl term
# (rel 1.8e-3) at 1.5x the PE cost of 2 terms.
W1TERMS = 2
W1SCALE = 64.0
# number of k-chunk pairs (of KD/2=4) whose x_lo correction is dropped
# (2 of 4: measured full-batch rel 1.65e-2 vs the 2e-2 gate; at 1 the
# measured rel is 1.50e-2, at 3 it projects over 1.9e-2)
XLO_SKIP = 2


class Cfg:
    def __init__(self, BL=4, NVS=(1152,) * 4, D=1024, H=4, F=512):
        # NVS: per-batch-slot padded valid-token count (batches are
        # assigned to slots sorted by count, so later slots can be shorter)
        self.BL, self.NVS, self.D, self.H, self.F = BL, tuple(NVS), D, H, F
        assert len(self.NVS) == BL
        self.NV = max(self.NVS)   # shipped tensor size
        self.HF = H * F
        self.KD = D // P          # k-chunks of D
        self.MC = self.HF // P    # f-chunks of H*F
        self.NCV = -(-self.NV // P)  # token chunks (max slot, ceil)
        self.KOUT = (H * D) // P  # k-chunks of the output projection

    def blocks(self, b):
        """DMA token blocks for batch slot b: 512-wide + remainder. A
        short (<=128) remainder is merged with 128 tokens borrowed from
        the previous block so the final chunk has two sub-blocks - the
        narrow one is interleave-paired with a full one to keep the PSUM
        buffer-recycle window wider than the tanh pipeline latency."""
        nv = self.NVS[b]
        out = [512] * (nv // 512)
        r = nv % 512
        if r:
            if r <= P and out:
                out[-1] -= P
                r += P
            out.append(r)
        return out


def choose_slots(valid_mask: np.ndarray, n_cores=8, BL=4):
    """Sort batches by valid count; slot i takes ranks [i*n_cores,
    (i+1)*n_cores) so each slot's NV covers its 8 batches exactly
    (partial trailing token chunks are fine - tokens ride free/K dims).
    Returns (NVS, order) with order[i*n_cores + c] = original batch
    index processed by core c in slot i."""
    cnt = np.asarray(valid_mask).sum(axis=1)
    order = np.argsort(-cnt, kind="stable")
    NVS = []
    for i in range(BL):
        grp = cnt[order[i * n_cores : (i + 1) * n_cores]]
        NVS.append(max(2, int(grp.max())))
    return tuple(NVS), order


def build_kernel(nc: bass.Bass, cfg: Cfg, reps: int = 1):
    c = cfg
    # x^T and W1 ship as fp8 hi (e4m3) + residual lo (e5m2); the h matmul
    # runs 3 split terms (xh.Wh + xl.Wh + xh.Wl) in DoubleRow mode, each
    # instruction contracting TWO 128-deep k-chunks at 0.5 cycles/row -
    # 0.75x the bf16 PE cost at better-than-bf16 accuracy (the dropped
    # xl.Wl term is ~2^-8 relative).
    xth_d = nc.dram_tensor("xth", [c.BL, c.KD, P, c.NV], F8H, kind="ExternalInput").ap()
    xtl_d = nc.dram_tensor("xtl", [c.BL, c.KD, P, c.NV], F8L, kind="ExternalInput").ap()
    xn_d = nc.dram_tensor("xn", [c.BL, c.NV, c.D], BF16, kind="ExternalInput").ap()
    v_d = nc.dram_tensor("v", [c.BL, P, c.NCV], BF16, kind="ExternalInput").ap()
    w1h_d = nc.dram_tensor("w1h", [c.KD, P, c.HF], F8H, kind="ExternalInput").ap()
    w1l_d = nc.dram_tensor("w1l", [c.KD, P, c.HF], F8L, kind="ExternalInput").ap()
    w2_d = nc.dram_tensor("w2", [c.MC, P, c.H], BF16, kind="ExternalInput").ap()
    b1_d = nc.dram_tensor("b1", [c.HF], FP32, kind="ExternalInput").ap()
    wout_d = nc.dram_tensor("wout", [c.KOUT, P, c.D], BF16, kind="ExternalInput").ap()
    bout_d = nc.dram_tensor("bout", [P, c.KD * c.BL], FP32, kind="ExternalInput").ap()
    y_d = nc.dram_tensor("y", [c.BL, c.D], FP32, kind="ExternalOutput").ap()

    with tile.TileContext(nc) as tc:
        with (
            tc.tile_pool(name="const", bufs=1) as const,
            tc.tile_pool(name="xT", bufs=4) as xT_pool,
            tc.tile_pool(name="h", bufs=4) as h_pool,
            tc.tile_pool(name="xn", bufs=3) as xn_pool,
            tc.tile_pool(name="e", bufs=10) as e_pool,
            tc.tile_pool(name="small", bufs=8) as small_pool,
            tc.tile_pool(name="ysb", bufs=1) as ysb_pool,
            tc.tile_pool(name="hps", bufs=3, space="PSUM") as hps_pool,
            tc.tile_pool(name="scps", bufs=4, space="PSUM") as scps_pool,
            tc.tile_pool(name="plps", bufs=1, space="PSUM") as plps_pool,
            tc.tile_pool(name="acc", bufs=2) as acc_pool,
        ):
            # ---- constants / weights ----
            # W1 as 4 column-quarter tiles: the first matmul group only
            # waits for quarter 0 (~1MB); the rest stream in behind it.
            QW = c.HF // 4
            QMC = c.MC // 4  # f-chunks per quarter
            w1qh = [
                const.tile([P, c.KD, QW], F8H, tag=f"w1qh{q}", name=f"w1qh{q}")
                for q in range(4)
            ]
            w1ql = [
                const.tile([P, c.KD, QW], F8L, tag=f"w1ql{q}", name=f"w1ql{q}")
                if W1TERMS >= 3 else None
                for q in range(4)
            ]

            def w1_piece(q, lo, eng=None):
                # one fp8 quarter (0.5MB, 512B innermost -> full DMA rate);
                # hi quarters stream on sync, lo on gpsimd, concurrently
                dst, src = (w1ql[q], w1l_d) if lo else (w1qh[q], w1h_d)
                (eng or nc.sync).dma_start(
                    dst[:],
                    src[:, :, ts(q, QW)].rearrange("k p f -> p k f"),
                )

            w1_piece(0, 0)
            w2_sb = const.tile([P, c.MC, c.H], BF16)
            b1_sb = const.tile([P, c.MC], FP32)
            v_sb = const.tile([P, c.BL, c.NCV], BF16)
            bout_sb = const.tile([P, c.KD * c.BL], FP32)
            # pooled rows at r = b*32 + h (32-aligned per batch for DVE)
            # pooled^T accumulates directly in [128 d, (dc, h)] tiles; the
            # out-projection picks per-(dc,h) batch columns from poolT_sb
            poolT_sb = const.tile([P, c.KD, c.BL, c.H], BF16)
            ones_f32 = const.tile([1, P], FP32)
            nc.gpsimd.memset(ones_f32[:], 1.0)
            wout_sb = const.tile([P, c.KOUT, c.D], BF16)

            pending = []  # deferred (other-block) op emitters

            def flush(limit=0):
                # keep `limit` closures queued: pool matmuls lag their
                # sub-block by a few positions so early ones never stall
                # the PE on the xn DMA
                while len(pending) > limit:
                    pending.pop(0)()

            # flat chunk schedule across batches; xT DMAs prefetch 2 ahead
            sched = []
            for b in range(c.BL):
                t0 = 0
                for bi, TB in enumerate(c.blocks(b)):
                    sched.append((b, bi, t0, TB))
                    t0 += TB
            xts = {}

            def ensure_xt(k):
                if k >= len(sched) or k in xts:
                    return
                b, bi, t0, TB = sched[k]
                xTh = xT_pool.tile([P, c.KD, 512], F8H, tag="xth", name=f"xTh{k % 4}")
                xTl = xT_pool.tile([P, c.KD, 512], F8L, tag="xtl", name=f"xTl{k % 4}")
                if k == 0:
                    # first xt tiles: hi on scalar, lo on gpsimd - both
                    # transfer concurrently with w1q0-hi on sync
                    nc.scalar.dma_start(
                        xTh[:, :, 0:TB],
                        xth_d[b, :, :, t0 : t0 + TB].rearrange("k p t -> p k t"),
                    )
                    nc.gpsimd.dma_start(
                        xTl[:, :, 0:TB],
                        xtl_d[b, :, :, t0 : t0 + TB].rearrange("k p t -> p k t"),
                    )
                    # small weights behind them on the scalar queue
                    nc.scalar.dma_start(
                        b1_sb[:], b1_d.rearrange("(c p) -> p c", p=P)
                    )
                    nc.scalar.dma_start(
                        w2_sb[:], w2_d.rearrange("c p h -> p c h")
                    )
                    nc.scalar.dma_start(
                        v_sb[:], v_d.rearrange("b p c -> p b c")
                    )
                    nc.scalar.dma_start(bout_sb[:], bout_d)
                    # rest of W1: hi quarters on sync, lo on gpsimd
                    if W1TERMS >= 3:
                        w1_piece(0, 1, nc.gpsimd)
                    for q in range(1, 4):
                        w1_piece(q, 0)
                        if W1TERMS >= 3:
                            w1_piece(q, 1, nc.gpsimd)
                else:
                    nc.sync.dma_start(
                        xTh[:, :, 0:TB],
                        xth_d[b, :, :, t0 : t0 + TB].rearrange("k p t -> p k t"),
                    )
                    nc.sync.dma_start(
                        xTl[:, :, 0:TB],
                        xtl_d[b, :, :, t0 : t0 + TB].rearrange("k p t -> p k t"),
                    )
                xts[k] = (xTh, xTl)

            kprev = -1
            for b in range(c.BL):
                # pl/z PSUM tiles are bufs=1: allocate only after the
                # previous batch's deferred consumers have been emitted
                # (first flush of this batch), so buffer-reuse tracking
                # sees ops in order.
                z_ps = acc_sb = None
                NCVb = -(-c.NVS[b] // P)
                for bi, TB in enumerate(c.blocks(b)):
                    kprev += 1
                    k = kprev
                    _, _, t0, _ = sched[k]
                    cn0 = t0 // P
                    first = k == 0 and TB == 512
                    ensure_xt(k)
                    ensure_xt(k + 1)
                    ensure_xt(k + 2)
                    ensure_xt(k + 3)
                    xTh, xTl = xts.pop(k)
                    # xn rides the (otherwise idle) gpsimd DMA queue so it
                    # never contends with the xt/wout stream on sync
                    xnt = xn_pool.tile([P, 4, c.D], BF16)
                    fullt = (TB // P) * P  # whole 128-token chunks
                    if fullt:
                        nc.gpsimd.dma_start(
                            xnt[:, 0 : TB // P, :],
                            xn_d[b, t0 : t0 + fullt, :].rearrange(
                                "(s p) d -> p s d", p=P
                            ),
                        )
                    if TB % P:
                        nc.gpsimd.dma_start(
                            xnt[0 : TB % P, TB // P, :],
                            xn_d[b, t0 + fullt : t0 + TB, :],
                        )
                    if b == 0 and bi == 1:
                        # prefetch the output projection during the middle
                        KQ = c.KOUT // 4
                        for q in range(4):
                            nc.sync.dma_start(
                                wout_sb[:, ts(q, KQ), :],
                                wout_d[ts(q, KQ)].rearrange("k p f -> p k f"),
                            )
                    # 128-token sub-blocks processed in PAIRS: the 3 fp8
                    # split terms accumulate per sub as sequential PSUM
                    # groups in one tile, then ONE tanh covers the pair
                    # (halving the Act engine's fixed access cost).
                    subs = -(-TB // P)
                    wid = [min(P, TB - s * P) for s in range(subs)]
                    groups = [
                        list(range(i, min(i + 4, subs)))
                        for i in range(0, subs, 4)
                    ]
                    for group in groups:
                        offs = {}
                        o = 0
                        for s in group:
                            offs[s] = o
                            o += wid[s]
                        wt = o  # total pair width
                        scs, prevq = {}, []
                        for s in group:
                            scs[s] = scps_pool.tile(
                                [P, 512], FP32, tag="sc_ps", name=f"sc{s}"
                            )

                        def dot(s, mc, h_sb):
                            nc.tensor.matmul(
                                scs[s][0 : wid[s], 0 : c.H],
                                h_sb[:, offs[s] : offs[s] + wid[s]],
                                w2_sb[:, mc, :],
                                start=(mc == 0),
                                stop=(mc == c.MC - 1),
                            )

                        for mc in range(c.MC):
                            h_ps = hps_pool.tile([P, 512], FP32, tag="h_ps")
                            wcol = (mc % QMC) * P
                            q = mc // QMC
                            terms = [(w1qh[q], xTh), (w1qh[q], xTl),
                                     (w1ql[q], xTh)][:W1TERMS]
                            for s in group:
                                w, off = wid[s], offs[s]
                                sp = s * P
                                for t, (lh, rh) in enumerate(terms):
                                    for pr in range(c.KD // 2):
                                        # the x_lo correction is skipped
                                        # for the first XLO_SKIP k-pairs:
                                        # adds x-quant error on 1/4 of the
                                        # contraction depth (measured rel
                                        # stays under the gate) for 12.5%
                                        # less PE work
                                        if t == 1 and pr < XLO_SKIP:
                                            continue
                                        dc = 2 * pr
                                        nc.tensor.matmul(
                                            h_ps[:, off : off + w],
                                            lh[:, dc : dc + 2, wcol : wcol + P],
                                            rh[:, dc : dc + 2, sp : sp + w],
                                            start=(t == 0 and pr == 0),
                                            stop=(
                                                t == len(terms) - 1
                                                and pr == c.KD // 2 - 1
                                            ),
                                            perf_mode=DR,
                                        )
                            h_sb = h_pool.tile([P, 4 * P], BF16, tag="h_sb")
                            nc.scalar.activation(
                                h_sb[:, 0:wt], h_ps[:, 0:wt], AFT.Tanh,
                                bias=b1_sb[:, mc : mc + 1],
                                scale=1.0 / W1SCALE,
                            )
                            prevq.append((mc, h_sb))
                            if len(prevq) > 2:
                                pm, ph = prevq.pop(0)
                                for s in group:
                                    dot(s, pm, ph)
                            if mc == 2:
                                # drain fully in the kernel's last chunk so
                                # deferred pool work doesn't pile into the tail
                                flush(0 if k == len(sched) - 1 else 3)
                        while prevq:
                            pm, ph = prevq.pop(0)
                            for s in group:
                                dot(s, pm, ph)
                        if acc_sb is None:
                            # full flush: the previous batch's pool/finish
                            # closures must be emitted before the acc
                            # buffer rotates to this batch
                            flush()
                            acc_sb = acc_pool.tile(
                                [P, c.KD * c.H + c.H], FP32, tag="acc"
                            )
                        for s in group:
                            w = wid[s]
                            e_blk = e_pool.tile([P, c.H], BF16, tag="e_blk")
                            nc.scalar.activation(
                                e_blk[0:w, :], scs[s][0:w, 0 : c.H],
                                AFT.Exp, bias=0.0,
                            )

                            def mk_pool(b, s, w, cn, ncv, e_blk, xnt, acc_sb):
                                def go():
                                    # pooled^T chunk: [128 d, H] per dc -
                                    # free dim 4, nearly free on the PE -
                                    # plus Z[h] = v . e on partition 0;
                                    # DVE accumulates chunks across cn
                                    NA = c.KD * c.H
                                    plw = plps_pool.tile(
                                        [P, 512], FP32, tag="plw", name="plw"
                                    )
                                    for dc in range(c.KD):
                                        nc.tensor.matmul(
                                            plw[:, dc * c.H : (dc + 1) * c.H],
                                            xnt[0:w, s, ts(dc, P)],
                                            e_blk[0:w, :],
                                            start=True,
                                            stop=True,
                                        )
                                    nc.tensor.matmul(
                                        plw[0:1, NA : NA + c.H],
                                        v_sb[0:w, b, cn : cn + 1],
                                        e_blk[0:w, :],
                                        start=True,
                                        stop=True,
                                    )
                                    if cn == 0:
                                        nc.vector.tensor_copy(
                                            acc_sb[:, 0:NA], plw[:, 0:NA]
                                        )
                                        nc.vector.tensor_copy(
                                            acc_sb[0:1, NA : NA + c.H],
                                            plw[0:1, NA : NA + c.H],
                                        )
                                    else:
                                        nc.vector.tensor_add(
                                            acc_sb[:, 0:NA], acc_sb[:, 0:NA],
                                            plw[:, 0:NA],
                                        )
                                        nc.vector.tensor_add(
                                            acc_sb[0:1, NA : NA + c.H],
                                            acc_sb[0:1, NA : NA + c.H],
                                            plw[0:1, NA : NA + c.H],
                                        )
                                return go

                            pending.append(
                                mk_pool(b, s, w, cn0 + s, NCVb, e_blk, xnt,
                                        acc_sb)
                            )
                    t0 += TB

                def mk_finish(b, acc_sb):
                    def go():
                        NA = c.KD * c.H
                        rz = small_pool.tile([1, c.H], FP32, tag="rz")
                        nc.vector.reciprocal(rz[:], acc_sb[0:1, NA : NA + c.H])
                        # broadcast rz down the partitions via the PE, then
                        # scale acc into poolT_sb one head-column at a time
                        rzb_ps = scps_pool.tile(
                            [P, 512], FP32, tag="sc_ps", name="rzb"
                        )
                        nc.tensor.matmul(
                            rzb_ps[:, 0 : c.H], ones_f32[:], rz[:],
                            start=True, stop=True,
                        )
                        rzb = small_pool.tile([P, c.H], FP32, tag="rzb")
                        nc.vector.tensor_copy(rzb[:], rzb_ps[:, 0 : c.H])
                        accv = acc_sb[:, 0:NA].rearrange(
                            "p (k h) -> p h k", h=c.H
                        )
                        for hd in range(c.H):
                            nc.vector.tensor_scalar_mul(
                                poolT_sb[:, :, b, hd],
                                accv[:, hd, :],
                                rzb[:, hd : hd + 1],
                            )
                    return go

                pending.append(mk_finish(b, acc_sb))

            flush()
            # ---- output projection y = pooled @ Wout + bout ----
            # y^T chunks [128 d_out, BL]: free dim = BL = 4, so the 256
            # matmuls cost the PE almost nothing (vs 64 x 512-free rows
            # the straight orientation would charge). Groups sequential
            # per d_out chunk -> one PSUM region suffices.
            yt_ps = plps_pool.tile([P, 512], FP32, tag="plw", name="yt_ps")
            ytcols = yt_ps[:, 0 : c.BL * c.KD].rearrange(
                "p (b o) -> p o b", o=c.KD
            )
            for do in range(c.KD):
                for hd in range(c.H):
                    for dc in range(c.KD):
                        nc.tensor.matmul(
                            ytcols[:, do, :],
                            wout_sb[:, hd * c.KD + dc, ts(do, P)],
                            poolT_sb[:, dc, :, hd],
                            start=(hd == 0 and dc == 0),
                            stop=(hd == c.H - 1 and dc == c.KD - 1),
                        )
            # += bout (pre-laid as [P, KD*BL]), then a strided DMA writes
            # y directly in [BL, D] order - no transposes, no extra copy
            yt_sb = ysb_pool.tile([P, c.KD * c.BL], FP32)
            nc.vector.tensor_add(
                yt_sb[:], yt_ps[:, 0 : c.KD * c.BL], bout_sb[:]
            )
            nc.sync.dma_start(
                y_d.rearrange("b (o p) -> p b o", p=P),
                yt_sb[:].rearrange("p (b o) -> p b o", o=c.KD),
            )
    return nc


def make_in_maps(x, valid_mask, W1, b1, W2, b2, Wout, bout, n_cores, cfg,
                 order=None):
    """Host-side prep: compact valid tokens, shard over batch, cast/layout."""
    c = cfg
    bf16 = ml_dtypes.bfloat16
    e4 = ml_dtypes.float8_e4m3
    e5 = ml_dtypes.float8_e5m2
    B = x.shape[0]
    # W1 split into fp8 hi + e5m2 residual (used in DoubleRow mode),
    # pre-scaled so its sigma sits well inside e4m3's normal range; the
    # tanh's scale undoes it exactly
    w1f = np.ascontiguousarray(
        W1.transpose(1, 0, 2).reshape(c.KD, P, c.HF).astype(np.float32)
        * np.float32(W1SCALE)
    )
    w1h_l = np.ascontiguousarray(w1f.astype(e4))
    w1l_l = np.ascontiguousarray(
        (w1f - w1h_l.astype(np.float32)).astype(e5)
    )
    w2f = W2.reshape(c.HF).astype(np.float32)
    w2_l = np.zeros((c.MC, P, c.H), np.float32)
    FC = c.MC // c.H  # f-chunks per head
    for mc in range(c.MC):
        w2_l[mc, :, mc // FC] = w2f[mc * P : (mc + 1) * P]
    w2_l = np.ascontiguousarray(w2_l.astype(bf16))
    b1_l = np.ascontiguousarray(b1.reshape(c.HF).astype(np.float32))
    wout_l = np.ascontiguousarray(Wout.reshape(c.KOUT, P, c.D).astype(bf16))
    # bout pre-laid for the y^T tail: bout_l[p, b*KD + do] = bout[do*128+p]
    bout_l = np.ascontiguousarray(
        np.tile(bout.astype(np.float32).reshape(c.KD, P).T, (1, c.BL))
    )
    # b2 is a per-row constant under the softmax -> it cancels; drop it.
    if order is None:
        order = np.arange(B)
    xc32 = np.zeros((B, c.NV, c.D), np.float32)
    v = np.zeros((B, c.NCV * P), np.float32)
    for gb in range(B):
        idx = np.flatnonzero(valid_mask[gb])[: c.NV]
        xc32[gb, : len(idx)] = x[gb, idx]
        v[gb, : len(idx)] = 1.0
    v_l = np.ascontiguousarray(
        v.reshape(B, c.NCV, P).transpose(0, 2, 1).astype(bf16)
    )
    xc = xc32.astype(bf16)  # pooling operand
    # x^T split into fp8 hi + e5m2 residual for the score matmul
    xt32 = np.ascontiguousarray(xc32.transpose(0, 2, 1)).reshape(
        B, c.KD, P, c.NV
    )
    xth_all = np.ascontiguousarray(xt32.astype(e4))
    xtl_all = np.ascontiguousarray(
        (xt32 - xth_all.astype(np.float32)).astype(e5)
    )
    in_maps = []
    for core in range(n_cores):
        sel = [order[i * n_cores + core] for i in range(c.BL)]
        in_maps.append(
            {
                "xth": np.ascontiguousarray(xth_all[sel]),
                "xtl": np.ascontiguousarray(xtl_all[sel]),
                "xn": np.ascontiguousarray(xc[sel]),
                "v": np.ascontiguousarray(v_l[sel]),
                "w1h": w1h_l,
                "w1l": w1l_l,
                "w2": w2_l,
                "b1": b1_l,
                "wout": wout_l,
                "bout": bout_l,
            }
        )
    return in_maps


_cached = {}
last_results = None


def kernel(x, valid_mask, W1, b1, W2, b2, Wout, bout, trace=False):
    global last_results
    x, valid_mask, W1, b1, W2, b2, Wout, bout = (
        np.asarray(a)
        for a in (x, valid_mask, W1, b1, W2, b2, Wout, bout)
    )
    B = x.shape[0]
    n_cores = 8
    NVS, order = choose_slots(valid_mask, n_cores, B // n_cores)
    cfg = Cfg(BL=B // n_cores, NVS=NVS)
    key = (B, NVS)
    if key not in _cached:
        nc = bacc.Bacc("TRN2", target_bir_lowering=False, debug=False)
        build_kernel(nc, cfg)
        nc.compile()
        _cached[key] = nc
    in_maps = make_in_maps(
        x, valid_mask, W1, b1, W2, b2, Wout, bout, n_cores, cfg, order
    )
    res = run_bass_kernel_spmd(
        _cached[key], in_maps, core_ids=list(range(n_cores)), trace=trace
    )
    last_results = res
    y = np.empty((B, x.shape[2]), np.float32)
    for core in range(n_cores):
        yc = np.asarray(res.results[core]["y"], np.float32)
        for i in range(cfg.BL):
            y[order[i * n_cores + core]] = yc[i]
    return y
